# revision 1
# baseline (speedup 1.0000x reference)
"""Trainium2 Bass kernel for nn_AttentionEncoder (6-layer dense transformer).

Strategy
--------
Data-parallel over batch: 16 sequences across 8 NeuronCores (2 per core), no
collectives.  Per core, each sequence's residual stream h lives in SBUF in
d-major layout ([HIDDEN, SEQ] as 8 tiles of [128, 512]) for the whole network;
weights stream from HBM.  Big matmuls run in bf16 (1 cycle/row, half the
LDWEIGHTS cost of fp32/fp32r via fast-weight-load, half the weight DMA, lower
PE power -> less clock throttling); the residual stream, psum accumulation and
norm statistics stay fp32 (norm reduce/broadcast matmuls use fp32r).

Layouts (d-major residual stream):
  - Q/K computed d-major [head*64, SEQ]; V computed token-major [SEQ, head*64]
    (xn is the stationary operand), so attention needs no transposes:
      scores^T [kt, qt] = K_h(kxm) @ Q_h(kxn)        (per head, no mask)
      exp via ACT (scores bounded ~3.3, no max-subtraction needed)
      sumexp    = ones[128,1](kxm) @ (E0+E1 / E2+E3) (PE partition reduce)
      att [e,qt]= Vtok_h(kxm) @ E(kxn), then * bcast(1/sumexp)
  - RMSNorm in d-major: sum(h^2) over partitions via ones-matmul, sqrt on ACT,
    reciprocal on DVE, broadcast over partitions via k=1 ones-matmul.
  - gamma (g1/g2) pre-folded into Wq/Wk/Wv/W1 on host; biases applied on-chip
    (bo/b2 fused into the residual add via scalar_tensor_tensor, b1 fused into
    the Gelu activation bias).
  - Embedding lookup: one-hot(acts) built on-chip (PE broadcast + is_equal with
    an iota constant), with the duration channel appended as a 33rd one-hot
    row, so h = W_emb^T @ onehot + pos in a single matmul per d-chunk.
  - RMSNorm sum-of-squares is fused into the loop that produces the residual
    tiles (Wo/FFN2/embedding), so only sqrt->reciprocal->broadcast sits on the
    norm critical path and the PE stays dense.
  - A post-pass splits multi-wait instructions into single-wait EventSemaphore
    prefixes (this container's walrus accepts one sync-wait per instruction).
  - Final [HIDDEN, SEQ] -> [SEQ, HIDDEN] via PE transposes, contiguous DMA out.
"""

import os
import sys

import numpy as np

N_LAYER = 6
N_HEAD = 16
HIDDEN = 1024
HEAD = HIDDEN // N_HEAD
FFWD = 2048
SEQ = 512
VOCAB = 32
BATCH = 16
N_CORES = 8
SEQ_PER_CORE = BATCH // N_CORES

P = 128
DC = HIDDEN // P   # 8 d-chunks
FC = FFWD // P     # 16 f-chunks
TC = SEQ // P      # 4 token-chunks


def _ensure_paths():
    for p in (
        "/opt/trn_rl_repo",
        "/root/.axon_site",
        "/root/.axon_site/_ro/trn_rl_repo",
        "/root/.axon_site/_ro/pypackages",
    ):
        if os.path.isdir(p) and p not in sys.path:
            sys.path.append(p)


def build_nc(gelu_mode="hw", split_waits=True):
    _ensure_paths()
    import concourse.bass as bass
    import concourse.tile as tile
    from concourse import mybir
    from concourse.masks import make_identity

    F32 = mybir.dt.float32
    F32R = mybir.dt.float32r
    BF16 = mybir.dt.bfloat16
    Act = mybir.ActivationFunctionType
    Alu = mybir.AluOpType

    def r(ap):
        return ap.bitcast(F32R)

    nc = bass.Bass("TRN2", target_bir_lowering=False, debug=False)

    x_d = nc.dram_tensor("x", [SEQ_PER_CORE, SEQ, 2], F32, kind="ExternalInput").ap()
    wemb_d = nc.dram_tensor("wemb", [VOCAB + 1, HIDDEN], BF16, kind="ExternalInput").ap()
    post_d = nc.dram_tensor("post", [DC, P, SEQ], F32, kind="ExternalInput").ap()
    iota_d = nc.dram_tensor("iota", [VOCAB, 1], F32, kind="ExternalInput").ap()
    wqk_d = nc.dram_tensor("wqk", [N_LAYER, 2, DC, P, DC, P], BF16, kind="ExternalInput").ap()
    wv_d = nc.dram_tensor("wv", [N_LAYER, DC, P, HIDDEN], BF16, kind="ExternalInput").ap()
    wo_d = nc.dram_tensor("wo", [N_LAYER, DC, P, DC, P], BF16, kind="ExternalInput").ap()
    w1_d = nc.dram_tensor("w1", [N_LAYER, FC, P, DC, P], BF16, kind="ExternalInput").ap()
    w2_d = nc.dram_tensor("w2", [N_LAYER, DC, P, FC, P], BF16, kind="ExternalInput").ap()
    bo_d = nc.dram_tensor("bo", [N_LAYER, P, DC], F32, kind="ExternalInput").ap()
    b1_d = nc.dram_tensor("b1", [N_LAYER, P, FC], F32, kind="ExternalInput").ap()
    b2_d = nc.dram_tensor("b2", [N_LAYER, P, DC], F32, kind="ExternalInput").ap()
    out_d = nc.dram_tensor("out", [SEQ_PER_CORE, SEQ, HIDDEN], F32, kind="ExternalOutput").ap()

    eps = float(np.finfo(np.float32).eps)
    scale = float(HEAD ** -0.5)

    from contextlib import ExitStack

    with tile.TileContext(nc) as tc:
        with ExitStack() as ctx:
            pool = lambda *a, **kw: ctx.enter_context(tc.tile_pool(*a, **kw))
            pc = pool(name="pc", bufs=1)
            pbias = pool(name="pbias", bufs=2)
            ph = pool(name="ph", bufs=12)
            pact = pool(name="pact", bufs=12)
            pq = pool(name="pq", bufs=9)
            pk = pool(name="pk", bufs=9)
            pv = pool(name="pv", bufs=6)
            pwv = pool(name="pwv", bufs=10)
            pE = pool(name="pE", bufs=12)
            pet = pool(name="pet", bufs=8)
            pg = pool(name="pg", bufs=18)
            pw = pool(name="pw", bufs=8)
            posb = pool(name="posb", bufs=2)
            psm = pool(name="psm", bufs=3)
            pp_mm = pool(name="pp_mm", bufs=4, space="PSUM")
            pp_att = pool(name="pp_att", bufs=3, space="PSUM")
            pp_red = pool(name="pp_red", bufs=1, space="PSUM")
            # constants (memset cannot write fp32r; stage via f32 + copy)
            ones_f = pc.tile([P, P], F32, name="ones_f")
            nc.vector.memset(ones_f, 1.0)
            ones_row = pc.tile([1, P], F32R, name="ones_row")
            nc.vector.tensor_copy(out=ones_row, in_=ones_f[0:1, :])
            ones_col = pc.tile([P, 1], F32R, name="ones_col")
            nc.vector.tensor_copy(out=ones_col, in_=ones_f[:, 0:1])
            ones_col_b = pc.tile([P, 1], BF16, name="ones_col_b")
            nc.vector.tensor_copy(out=ones_col_b, in_=ones_f[:, 0:1])
            ones_row_b = pc.tile([1, P], BF16, name="ones_row_b")
            nc.vector.tensor_copy(out=ones_row_b, in_=ones_f[0:1, :])
            ident = pc.tile([P, P], F32, name="ident")
            make_identity(nc, ident)
            iota_t = pc.tile([VOCAB, 1], F32, name="iota_t")
            nc.sync.dma_start(out=iota_t, in_=iota_d)
            eps_t = pc.tile([1, 1], F32, name="eps_t")
            nc.vector.memset(eps_t, eps)
            zero_col = pc.tile([P, 1], F32, name="zero_col")
            nc.vector.memset(zero_col, 0.0)
            wemb_sb = pc.tile([VOCAB + 1, HIDDEN], BF16, name="wemb_sb")
            nc.sync.dma_start(out=wemb_sb, in_=wemb_d)

            def sumsq_start(nm):
                return pp_red.tile([1, SEQ], F32, tag="red", name=f"{nm}_ss")

            def sumsq_add(ps_ss, t, idx, nm):
                sq = pet.tile([P, SEQ], F32R, tag="et", name=f"{nm}_sq{idx}")
                nc.vector.tensor_mul(sq, t, t)
                nc.tensor.matmul(ps_ss, r(ones_col), r(sq),
                                 start=(idx == 0), stop=(idx == DC - 1))

            def rmsnorm_fin(h_tiles, ps_ss, nm):
                ss = psm.tile([1, SEQ], F32, tag="stat", name=f"{nm}_rms")
                nc.scalar.activation(out=ss, in_=ps_ss, func=Act.Sqrt,
                                     scale=1.0 / HIDDEN, bias=eps_t)
                inv = psm.tile([1, SEQ], F32R, tag="stat", name=f"{nm}_inv")
                with nc.allow_low_precision(reason="fp32r is 32-bit storage"):
                    nc.vector.reciprocal(out=inv, in_=ss)
                ps_b = pp_att.tile([P, SEQ], F32, tag="att", name=f"{nm}_bc")
                nc.tensor.matmul(ps_b, r(ones_row), r(inv), start=True, stop=True)
                xn = []
                for kc in range(DC):
                    xt = pact.tile([P, SEQ], BF16, tag="act", name=f"{nm}_xn{kc}")
                    nc.vector.tensor_mul(xt, h_tiles[kc], ps_b)
                    xn.append(xt)
                return xn

            for s in range(SEQ_PER_CORE):
                # ---------------- embedding ----------------
                acts_f = psm.tile([1, SEQ], F32, tag="row", name=f"s{s}_actsf")
                nc.sync.dma_start(out=acts_f, in_=x_d[s:s + 1, :, 0])
                acts = psm.tile([1, SEQ], BF16, tag="row", name=f"s{s}_acts")
                nc.vector.tensor_copy(out=acts, in_=acts_f)
                dur = psm.tile([1, SEQ], F32, tag="row", name=f"s{s}_dur")
                nc.sync.dma_start(out=dur, in_=x_d[s:s + 1, :, 1])
                ps_ab = pp_att.tile([VOCAB, SEQ], F32, tag="att", name=f"s{s}_ab")
                nc.tensor.matmul(ps_ab, ones_row_b[:, :VOCAB], acts,
                                 start=True, stop=True)
                oh = psm.tile([VOCAB + 1, SEQ], BF16, tag="oh", name=f"s{s}_oh")
                nc.vector.tensor_scalar(out=oh[0:VOCAB, :], in0=ps_ab,
                                        scalar1=iota_t, scalar2=None,
                                        op0=Alu.is_equal)
                nc.vector.tensor_copy(out=oh[VOCAB:VOCAB + 1, :], in_=dur)

                h = []
                ss_next = sumsq_start(f"s{s}emb")
                for mc in range(DC):
                    ps = pp_mm.tile([P, SEQ], F32, tag="mm", name=f"s{s}_emb{mc}")
                    nc.tensor.matmul(ps, wemb_sb[:, mc * P:(mc + 1) * P], oh,
                                     start=True, stop=True)
                    pos_t = pact.tile([P, SEQ], F32, tag="act", name=f"s{s}_pos{mc}")
                    nc.sync.dma_start(out=pos_t, in_=post_d[mc])
                    hm = ph.tile([P, SEQ], F32, tag="h", name=f"s{s}_h{mc}")
                    nc.vector.tensor_add(hm, ps, pos_t)
                    sumsq_add(ss_next, hm, mc, f"s{s}emb")
                    h.append(hm)

                for li in range(N_LAYER):
                    nm = f"s{s}l{li}"
                    bo_sb = pbias.tile([P, DC], F32, tag="bo", name=f"{nm}_bo")
                    nc.sync.dma_start(out=bo_sb, in_=bo_d[li])
                    b1_sb = pbias.tile([P, FC], F32, tag="b1", name=f"{nm}_b1")
                    nc.sync.dma_start(out=b1_sb, in_=b1_d[li])
                    b2_sb = pbias.tile([P, DC], F32, tag="b2", name=f"{nm}_b2")
                    nc.sync.dma_start(out=b2_sb, in_=b2_d[li])

                    # ---------------- attention ----------------
                    xn = rmsnorm_fin(h, ss_next, nm + "n1")

                    qk = []
                    for t in range(2):
                        dst = []
                        pool = pq if t == 0 else pk
                        for mc in range(DC):
                            wt = pw.tile([P, DC, P], BF16, tag="w", name=f"{nm}_wqk{t}_{mc}")
                            nc.sync.dma_start(out=wt, in_=wqk_d[li, t, mc])
                            ps = pp_mm.tile([P, SEQ], F32, tag="mm", name=f"{nm}_qk{t}{mc}")
                            for kc in range(DC):
                                nc.tensor.matmul(ps, wt[:, kc, :], xn[kc],
                                                 start=(kc == 0), stop=(kc == DC - 1))
                            dt_ = pool.tile([P, SEQ], BF16, tag="qk", name=f"{nm}_t{t}{mc}")
                            nc.vector.tensor_copy(out=dt_, in_=ps)
                            dst.append(dt_)
                        qk.append(dst)
                    q_tiles, k_tiles = qk

                    v_tiles = [pv.tile([P, HIDDEN], BF16, tag="v", name=f"{nm}_v{mc}")
                               for mc in range(TC)]
                    for nh in range(2):
                        wv_t = []
                        for kc in range(DC):
                            wvt = pwv.tile([P, 512], BF16, tag="wv", name=f"{nm}_wv{nh}_{kc}")
                            nc.sync.dma_start(out=wvt, in_=wv_d[li, kc, :, nh * 512:(nh + 1) * 512])
                            wv_t.append(wvt)
                        for mc in range(TC):
                            ps = pp_mm.tile([P, 512], F32, tag="mm", name=f"{nm}_v{nh}{mc}")
                            for kc in range(DC):
                                nc.tensor.matmul(ps, xn[kc][:, mc * P:(mc + 1) * P],
                                                 wv_t[kc],
                                                 start=(kc == 0), stop=(kc == DC - 1))
                            nc.vector.tensor_copy(
                                out=v_tiles[mc][:, nh * 512:(nh + 1) * 512], in_=ps)

                    att_tiles = [pact.tile([P, SEQ], BF16, tag="act", name=f"{nm}_at{mc}")
                                 for mc in range(DC)]
                    for hh in range(N_HEAD):
                        ti = hh // 2
                        po = (hh % 2) * HEAD
                        kt = k_tiles[ti]
                        qt = q_tiles[ti]
                        Eh = []
                        for mc in range(TC):
                            ps_s = pp_mm.tile([P, SEQ], F32, tag="mm", name=f"{nm}_s{hh}_{mc}")
                            nc.tensor.matmul(ps_s,
                                             kt[po:po + HEAD, mc * P:(mc + 1) * P],
                                             qt[po:po + HEAD, :],
                                             start=True, stop=True)
                            e = pE.tile([P, SEQ], BF16, tag="E", name=f"{nm}_e{hh}_{mc}")
                            nc.scalar.activation(out=e, in_=ps_s, func=Act.Exp,
                                                 scale=scale, bias=zero_col)
                            Eh.append(e)
                        tmp1 = pet.tile([P, SEQ], BF16, tag="et", name=f"{nm}_t1_{hh}")
                        nc.vector.tensor_add(tmp1, Eh[0], Eh[1])
                        tmp2 = pet.tile([P, SEQ], BF16, tag="et", name=f"{nm}_t2_{hh}")
                        nc.vector.tensor_add(tmp2, Eh[2], Eh[3])
                        ps_sum = pp_red.tile([1, SEQ], F32, tag="red", name=f"{nm}_se{hh}")
                        nc.tensor.matmul(ps_sum, ones_col_b, tmp1, start=True, stop=False)
                        nc.tensor.matmul(ps_sum, ones_col_b, tmp2, start=False, stop=True)
                        rcp = psm.tile([1, SEQ], F32R, tag="stat", name=f"{nm}_rc{hh}")
                        with nc.allow_low_precision(reason="fp32r is 32-bit storage"):
                            nc.vector.reciprocal(out=rcp, in_=ps_sum)
                        ps_rb = pp_att.tile([HEAD, SEQ], F32, tag="att", name=f"{nm}_rb{hh}")
                        nc.tensor.matmul(ps_rb, r(ones_row[:, :HEAD]), r(rcp),
                                         start=True, stop=True)
                        rb = psm.tile([HEAD, SEQ], F32, tag="rb", name=f"{nm}_rbs{hh}")
                        nc.scalar.copy(out=rb, in_=ps_rb)
                        ps_a = pp_att.tile([HEAD, SEQ], F32, tag="att", name=f"{nm}_a{hh}")
                        for mc in range(TC):
                            nc.tensor.matmul(ps_a,
                                             v_tiles[mc][:, hh * HEAD:(hh + 1) * HEAD],
                                             Eh[mc],
                                             start=(mc == 0), stop=(mc == TC - 1))
                        nc.vector.tensor_mul(att_tiles[ti][po:po + HEAD, :], ps_a, rb)

                    # Wo + residual
                    ss_mid = sumsq_start(nm + "mid")
                    h2 = []
                    for mc in range(DC):
                        wt = pw.tile([P, DC, P], BF16, tag="w", name=f"{nm}_wo{mc}")
                        nc.sync.dma_start(out=wt, in_=wo_d[li, mc])
                        ps = pp_mm.tile([P, SEQ], F32, tag="mm", name=f"{nm}_o{mc}")
                        for kc in range(DC):
                            nc.tensor.matmul(ps, wt[:, kc, :], att_tiles[kc],
                                             start=(kc == 0), stop=(kc == DC - 1))
                        hn = ph.tile([P, SEQ], F32, tag="h", name=f"{nm}_h2{mc}")
                        nc.vector.scalar_tensor_tensor(
                            out=hn, in0=ps, scalar=bo_sb[:, mc:mc + 1], in1=h[mc],
                            op0=Alu.add, op1=Alu.add)
                        sumsq_add(ss_mid, hn, mc, nm + "mid")
                        h2.append(hn)
                    h = h2

                    # ---------------- FFN ----------------
                    yn = rmsnorm_fin(h, ss_mid, nm + "n2")
                    g_tiles = []
                    for mc in range(FC):
                        wt = pw.tile([P, DC, P], BF16, tag="w", name=f"{nm}_w1{mc}")
                        nc.sync.dma_start(out=wt, in_=w1_d[li, mc])
                        ps = pp_mm.tile([P, SEQ], F32, tag="mm", name=f"{nm}_f1{mc}")
                        for kc in range(DC):
                            nc.tensor.matmul(ps, wt[:, kc, :], yn[kc],
                                             start=(kc == 0), stop=(kc == DC - 1))
                        g = pg.tile([P, SEQ], BF16, tag="g", name=f"{nm}_g{mc}")
                        gelu_fn = Act.Gelu if gelu_mode == "hw" else Act.Identity
                        nc.scalar.activation(out=g, in_=ps, func=gelu_fn,
                                             bias=b1_sb[:, mc:mc + 1], scale=1.0)
                        g_tiles.append(g)

                    h3 = []
                    if li < N_LAYER - 1:
                        ss_next = sumsq_start(nm + "nxt")
                    for mc in range(DC):
                        wt = pw.tile([P, FC, P], BF16, tag="w", name=f"{nm}_w2{mc}")
                        nc.sync.dma_start(out=wt, in_=w2_d[li, mc])
                        ps = pp_mm.tile([P, SEQ], F32, tag="mm", name=f"{nm}_f2{mc}")
                        for kc in range(FC):
                            nc.tensor.matmul(ps, wt[:, kc, :], g_tiles[kc],
                                             start=(kc == 0), stop=(kc == FC - 1))
                        hn = ph.tile([P, SEQ], F32, tag="h", name=f"{nm}_h3{mc}")
                        nc.vector.scalar_tensor_tensor(
                            out=hn, in0=ps, scalar=b2_sb[:, mc:mc + 1], in1=h[mc],
                            op0=Alu.add, op1=Alu.add)
                        if li < N_LAYER - 1:
                            sumsq_add(ss_next, hn, mc, nm + "nxt")
                        h3.append(hn)
                    h = h3

                # ---------------- transpose + store ----------------
                for tck in range(TC):
                    ob = posb.tile([P, HIDDEN], F32, tag="osb", name=f"s{s}_ob{tck}")
                    for dc in range(DC):
                        ps_t = pp_mm.tile([P, P], F32, tag="mm", name=f"s{s}_tr{tck}_{dc}")
                        nc.tensor.transpose(ps_t, h[dc][:, tck * P:(tck + 1) * P], ident)
                        nc.vector.tensor_copy(out=ob[:, dc * P:(dc + 1) * P], in_=ps_t)
                    nc.sync.dma_start(out=out_d[s, tck * P:(tck + 1) * P, :], in_=ob)

    if split_waits:
        _split_multiwait(nc)
    return nc


def _split_multiwait(nc, max_waits=1):
    """This container's walrus accepts at most one sync-wait per instruction;
    hoist excess waits onto standalone EventSemaphore ops on the same engine
    queue (queue order preserves semantics)."""
    import bass_rust
    from bass_rust import SyncInfo

    for fn in nc.m.functions:
        for blk in fn.blocks:
            out = []
            for inst in blk.instructions:
                si = inst.sync_info
                waits = list(si.on_wait) if si is not None and si.on_wait else []
                if len(waits) > max_waits:
                    extra, keep = waits[:-max_waits], waits[-max_waits:]
                    for i, w in enumerate(extra):
                        nop = bass_rust.InstEventSemaphore(
                            name=f"{inst.name}w{i}", engine=inst.engine)
                        nop.sync_info = SyncInfo(on_wait=[w], on_update=[])
                        out.append(nop)
                    inst.sync_info = SyncInfo(
                        on_wait=keep, on_update=list(si.on_update or []))
                out.append(inst)
            blk.instructions = out


def prep_inputs(inputs):
    """Host-side layout prep shared by all cores (weights identical per core)."""
    _ensure_paths()
    import ml_dtypes

    f32 = np.float32
    emb = np.asarray(inputs["emb_table"], f32)       # [32, 1023]
    pos = np.asarray(inputs["pos_table"], f32)       # [512, 1024]
    Wq = np.asarray(inputs["Wq"], f32)               # [6, 16, 1024, 64]
    Wk = np.asarray(inputs["Wk"], f32)
    Wv = np.asarray(inputs["Wv"], f32)
    Wo = np.asarray(inputs["Wo"], f32)               # [6, 1024, 1024]
    W1 = np.asarray(inputs["W1"], f32)               # [6, 1024, 2048]
    W2 = np.asarray(inputs["W2"], f32)               # [6, 2048, 1024]
    g1 = np.asarray(inputs["g1"], f32)               # [6, 1024]
    g2 = np.asarray(inputs["g2"], f32)

    wemb = np.zeros((VOCAB + 1, HIDDEN), ml_dtypes.bfloat16)
    wemb[:VOCAB, :HIDDEN - 1] = emb.astype(ml_dtypes.bfloat16)
    wemb[VOCAB, HIDDEN - 1] = 1.0                    # duration channel

    post = np.ascontiguousarray(pos.T.reshape(DC, P, SEQ))
    iota = np.arange(VOCAB, dtype=f32).reshape(VOCAB, 1)

    def blk_kxm(a, mchunks):
        # [K, M] -> [mc, p, kc, m] blocked for contiguous per-partition DMA
        k, m = a.shape
        return np.ascontiguousarray(
            a.reshape(k // P, P, mchunks, P).transpose(2, 1, 0, 3))

    bf16 = ml_dtypes.bfloat16
    wqk = np.empty((N_LAYER, 2, DC, P, DC, P), bf16)
    wv = np.empty((N_LAYER, DC, P, HIDDEN), bf16)
    wo = np.empty((N_LAYER, DC, P, DC, P), bf16)
    w1 = np.empty((N_LAYER, FC, P, DC, P), bf16)
    w2 = np.empty((N_LAYER, DC, P, FC, P), bf16)
    for i in range(N_LAYER):
        aq = (Wq[i] * g1[i][None, :, None]).transpose(1, 0, 2).reshape(HIDDEN, HIDDEN)
        ak = (Wk[i] * g1[i][None, :, None]).transpose(1, 0, 2).reshape(HIDDEN, HIDDEN)
        av = (Wv[i] * g1[i][None, :, None]).transpose(1, 0, 2).reshape(HIDDEN, HIDDEN)
        wqk[i, 0] = blk_kxm(aq, DC).astype(bf16)
        wqk[i, 1] = blk_kxm(ak, DC).astype(bf16)
        wv[i] = av.reshape(DC, P, HIDDEN).astype(bf16)
        wo[i] = blk_kxm(Wo[i], DC).astype(bf16)
        w1[i] = blk_kxm(g2[i][:, None] * W1[i], FC).astype(bf16)
        w2[i] = blk_kxm(W2[i], DC).astype(bf16)

    base = {
        "wemb": wemb, "post": post, "iota": iota,
        "wqk": wqk, "wv": wv, "wo": wo, "w1": w1, "w2": w2,
        "bo": np.ascontiguousarray(
            np.asarray(inputs["bo"], f32).reshape(N_LAYER, DC, P).transpose(0, 2, 1)),
        "b1": np.ascontiguousarray(
            np.asarray(inputs["b1"], f32).reshape(N_LAYER, FC, P).transpose(0, 2, 1)),
        "b2": np.ascontiguousarray(
            np.asarray(inputs["b2"], f32).reshape(N_LAYER, DC, P).transpose(0, 2, 1)),
    }
    return base


LAST_RESULTS = None


def _ntff_hook():
    """NTFF profiling hook via the axon .so (the concourse<->antenv bridge
    module is absent in this image, so drive the capture directly)."""
    try:
        from trn_agent_boot.trn_boot import _ntff_profile_via_ctypes
        return _ntff_profile_via_ctypes("/opt/axon/libaxon_pjrt.so")
    except Exception as e:
        print("ntff hook unavailable:", e)
        return None


def kernel(**inputs):
    global LAST_RESULTS
    _ensure_paths()
    from concourse.bass_utils import run_bass_kernel_spmd

    x = np.asarray(inputs["x"], np.float32)          # [16, 512, 2]
    base = prep_inputs(inputs)
    in_maps = []
    for c in range(N_CORES):
        m = dict(base)
        m["x"] = np.ascontiguousarray(x[c * SEQ_PER_CORE:(c + 1) * SEQ_PER_CORE])
        in_maps.append(m)

    nc = build_nc()
    trace_dir = os.environ.get("KBENCH_TRACE_DIR")
    if trace_dir:
        hook = _ntff_hook()
        if hook is not None:
            os.makedirs(trace_dir, exist_ok=True)
            with hook(trace_dir, [0]):
                res = run_bass_kernel_spmd(nc, in_maps, list(range(N_CORES)))
        else:
            res = run_bass_kernel_spmd(nc, in_maps, list(range(N_CORES)))
    else:
        res = run_bass_kernel_spmd(nc, in_maps, list(range(N_CORES)))
    LAST_RESULTS = res
    out = np.concatenate(
        [res.results[c]["out"].reshape(SEQ_PER_CORE, SEQ * HIDDEN)
         for c in range(N_CORES)], axis=0)
    return out



# revision 7
# speedup vs baseline: 1.4312x; 1.4312x over previous
"""Trainium2 Bass kernel for nn_AttentionEncoder (6-layer dense transformer).

Strategy (v2)
-------------
Data-parallel over batch: 16 sequences across 8 NeuronCores (2 per core), no
collectives.  Per core, each sequence's residual stream h lives in SBUF in
d-major layout ([HIDDEN, SEQ] as 8 tiles of [128, 512]); weights stream from
HBM in bf16; psum accumulation fp32.

v2 changes vs v1 (3.17ms):
  - The two sequences per core run as generator-interleaved instruction
    streams with a half-layer phase offset, so while one sequence is in its
    ACT-heavy attention phase the other feeds the PE dense FFN/QKV matmuls.
    This keeps TensorE busy (no 1-3us gaps) and therefore HAM-warm (2.4GHz
    instead of oscillating down to 1.2GHz, which alone was ~40% of v1 time).
  - Q/K are computed per head-pair right before that pair's attention
    (chunk mc of the d-major Q/K output == head pair mc), cutting q/k SBUF
    lifetime ~10x.
  - Softmax: per head-pair, sum+broadcast fused into ones[128,64]-stationary
    matmuls accumulating into one [128,512] psum bank (rows 0:64 = head h
    sumexp broadcast, 64:128 = head h'); one DVE reciprocal_approx_fast per
    pair (replaces v1's 3.3us serial [1,512] DVE reciprocal per head); one
    DVE multiply normalizes both heads into the d-major att tile.
  - Attention matmuls use base-partition slices so the scores of a head pair
    land on PE row-groups 0/64 and AV+bcast on col-groups 0/64, letting the
    16x 32x32 sub-array structure run both heads' matmuls concurrently.
  - RMSNorm: ACT Sqrt -> DVE reciprocal_approx_fast -> ones-row broadcast
    matmul (no serial DVE reciprocal).
  - Psum->SBUF evacuations moved to the Scalar engine (Copy needs no
    activation-table reload); DVE keeps the residual/normalize multiplies.
  - A post-pass splits multi-wait instructions into single-wait
    EventSemaphore prefixes (this container's walrus accepts one sync-wait
    per instruction).
"""

import os
import sys

import numpy as np

N_LAYER = 6
N_HEAD = 16
HIDDEN = 1024
HEAD = HIDDEN // N_HEAD
FFWD = 2048
SEQ = 512
VOCAB = 32
BATCH = 16
N_CORES = 8
SEQ_PER_CORE = BATCH // N_CORES

P = 128
DC = HIDDEN // P   # 8 d-chunks == 8 head pairs
FC = FFWD // P     # 16 f-chunks
TC = SEQ // P      # 4 token-chunks

OFFSET = 36        # units to prime seq 0 ahead of seq 1 (~half a layer)


def _ensure_paths():
    for p in (
        "/opt/trn_rl_repo",
        "/root/.axon_site",
        "/root/.axon_site/_ro/trn_rl_repo",
        "/root/.axon_site/_ro/pypackages",
    ):
        if os.path.isdir(p) and p not in sys.path:
            sys.path.append(p)


def build_nc(split_waits=True):
    _ensure_paths()
    import concourse.bass as bass
    import concourse.tile as tile
    from concourse import mybir
    from concourse.masks import make_identity

    F32 = mybir.dt.float32
    F32R = mybir.dt.float32r
    BF16 = mybir.dt.bfloat16
    Act = mybir.ActivationFunctionType
    Alu = mybir.AluOpType

    def r(ap):
        return ap.bitcast(F32R)

    nc = bass.Bass("TRN2", target_bir_lowering=False, debug=False)

    x_d = nc.dram_tensor("x", [SEQ_PER_CORE, SEQ, 2], F32, kind="ExternalInput").ap()
    wemb_d = nc.dram_tensor("wemb", [VOCAB + 1, HIDDEN], BF16, kind="ExternalInput").ap()
    post_d = nc.dram_tensor("post", [DC, P, SEQ], F32, kind="ExternalInput").ap()
    iota_d = nc.dram_tensor("iota", [VOCAB, 1], F32, kind="ExternalInput").ap()
    wqk_d = nc.dram_tensor("wqk", [N_LAYER, 2, DC, P, DC, P], BF16, kind="ExternalInput").ap()
    wv_d = nc.dram_tensor("wv", [N_LAYER, DC, P, HIDDEN], BF16, kind="ExternalInput").ap()
    wo_d = nc.dram_tensor("wo", [N_LAYER, DC, P, DC, P], BF16, kind="ExternalInput").ap()
    w1_d = nc.dram_tensor("w1", [N_LAYER, FC, P, DC, P], BF16, kind="ExternalInput").ap()
    w2_d = nc.dram_tensor("w2", [N_LAYER, DC, P, FC, P], BF16, kind="ExternalInput").ap()
    bo_d = nc.dram_tensor("bo", [N_LAYER, P, DC], F32, kind="ExternalInput").ap()
    b1_d = nc.dram_tensor("b1", [N_LAYER, P, FC], F32, kind="ExternalInput").ap()
    b2_d = nc.dram_tensor("b2", [N_LAYER, P, DC], F32, kind="ExternalInput").ap()
    out_d = nc.dram_tensor("out", [SEQ_PER_CORE, SEQ, HIDDEN], F32, kind="ExternalOutput").ap()

    eps = float(np.finfo(np.float32).eps)
    scale = float(HEAD ** -0.5)

    from contextlib import ExitStack

    with tile.TileContext(nc) as tc:
        with ExitStack() as ctx:
            pool = lambda *a, **kw: ctx.enter_context(tc.tile_pool(*a, **kw))
            pc = pool(name="pc", bufs=1)
            pst = pool(name="pst", bufs=3)
            ph = pool(name="ph", bufs=10)        # residual h, per-seq tag
            pxn = pool(name="pxn", bufs=18)      # xn/yn bf16
            pqk = pool(name="pqk", bufs=8)       # q/k pair tiles
            pv = pool(name="pv", bufs=8)         # v tiles [P, HIDDEN]
            pE = pool(name="pE", bufs=10)        # exp(scores)
            ptmp = pool(name="ptmp", bufs=5)     # E pair sums
            psq = pool(name="psq", bufs=4)       # h^2 for sumsq
            patt = pool(name="patt", bufs=9)    # attention output d-major
            pg = pool(name="pg", bufs=17)        # gelu outputs
            prb = pool(name="prb", bufs=2)       # softmax reciprocal bcast
            pw = pool(name="pw", bufs=5)         # weight chunks
            pbias = pool(name="pbias", bufs=3)
            ppos = pool(name="ppos", bufs=2)
            posb = pool(name="posb", bufs=2)
            pp_mm = pool(name="pp_mm", bufs=4, space="PSUM")
            pp_pair = pool(name="pp_pair", bufs=2, space="PSUM")
            pp_red = pool(name="pp_red", bufs=2, space="PSUM")

            # constants (memset cannot write fp32r; stage via f32 + copy)
            ones_f = pc.tile([P, P], F32, name="ones_f")
            nc.vector.memset(ones_f, 1.0)
            ones_row = pc.tile([1, P], F32R, name="ones_row")
            nc.vector.tensor_copy(out=ones_row, in_=ones_f[0:1, :])
            ones_col = pc.tile([P, 1], F32R, name="ones_col")
            nc.vector.tensor_copy(out=ones_col, in_=ones_f[:, 0:1])
            ones64_b = pc.tile([P, HEAD], BF16, name="ones64_b")
            nc.vector.tensor_copy(out=ones64_b, in_=ones_f[:, 0:HEAD])
            ones_row_b = pc.tile([1, P], BF16, name="ones_row_b")
            nc.vector.tensor_copy(out=ones_row_b, in_=ones_f[0:1, :])
            ident = pc.tile([P, P], F32, name="ident")
            make_identity(nc, ident)
            iota_t = pc.tile([VOCAB, 1], F32, name="iota_t")
            nc.sync.dma_start(out=iota_t, in_=iota_d)
            eps_t = pc.tile([1, 1], F32, name="eps_t")
            nc.vector.memset(eps_t, eps)
            zero_col = pc.tile([P, 1], F32, name="zero_col")
            nc.vector.memset(zero_col, 0.0)
            wemb_sb = pc.tile([VOCAB + 1, HIDDEN], BF16, name="wemb_sb")
            nc.sync.dma_start(out=wemb_sb, in_=wemb_d)

            class SumSq:
                """Accumulate sum over d of h^2 into a [1,SEQ] psum row."""

                def __init__(self, nm):
                    self.ps = pp_red.tile([1, SEQ], F32, tag="red", name=f"{nm}_ss")
                    self.started = False
                    self.n = 0

                def add(self, t, nm, total=DC):
                    sq = psq.tile([P, SEQ], F32R, tag="sq", name=f"{nm}_sq{self.n}")
                    nc.vector.tensor_mul(sq, t, t)
                    self.n += 1
                    nc.tensor.matmul(self.ps, r(ones_col), r(sq),
                                     start=not self.started,
                                     stop=(self.n == total))
                    self.started = True

            def norm_fin(h_tiles, ss, nm):
                rms = pst.tile([1, SEQ], F32, tag="stat", name=f"{nm}_rms")
                nc.scalar.activation(out=rms, in_=ss.ps, func=Act.Sqrt,
                                     scale=1.0 / HIDDEN, bias=eps_t)
                inv = pst.tile([1, SEQ], F32, tag="stat", name=f"{nm}_inv")
                nc.vector.reciprocal_approx_fast(out=inv, in_=rms)
                inv_r = pst.tile([1, SEQ], F32R, tag="stat", name=f"{nm}_invr")
                nc.vector.tensor_copy(out=inv_r, in_=inv)
                ps_b = pp_mm.tile([P, SEQ], F32, tag="mm", name=f"{nm}_bc")
                nc.tensor.matmul(ps_b, r(ones_row), inv_r, start=True, stop=True)
                xn = []
                for kc in range(DC):
                    xt = pxn.tile([P, SEQ], BF16, tag="xn", name=f"{nm}_xn{kc}")
                    nc.vector.tensor_mul(xt, h_tiles[kc], ps_b)
                    xn.append(xt)
                return xn

            def seq_program(s):
                nm0 = f"s{s}"
                # ---------------- embedding ----------------
                acts_f = pst.tile([1, SEQ], F32, tag="row", name=f"{nm0}_actsf")
                nc.sync.dma_start(out=acts_f, in_=x_d[s:s + 1, :, 0])
                acts = pst.tile([1, SEQ], BF16, tag="row", name=f"{nm0}_acts")
                nc.vector.tensor_copy(out=acts, in_=acts_f)
                dur = pst.tile([1, SEQ], F32, tag="row", name=f"{nm0}_dur")
                nc.sync.dma_start(out=dur, in_=x_d[s:s + 1, :, 1])
                ps_ab = pp_mm.tile([VOCAB, SEQ], F32, tag="mm", name=f"{nm0}_ab")
                nc.tensor.matmul(ps_ab, ones_row_b[:, :VOCAB], acts,
                                 start=True, stop=True)
                oh = pst.tile([VOCAB + 1, SEQ], BF16, tag="oh", bufs=2, name=f"{nm0}_oh")
                nc.vector.tensor_scalar(out=oh[0:VOCAB, :], in0=ps_ab,
                                        scalar1=iota_t, scalar2=None,
                                        op0=Alu.is_equal)
                nc.vector.tensor_copy(out=oh[VOCAB:VOCAB + 1, :], in_=dur)
                yield

                h = []
                ss_next = SumSq(f"{nm0}emb")
                for mc in range(DC):
                    ps = pp_mm.tile([P, SEQ], F32, tag="mm", name=f"{nm0}_emb{mc}")
                    nc.tensor.matmul(ps, wemb_sb[:, mc * P:(mc + 1) * P], oh,
                                     start=True, stop=True)
                    pos_t = ppos.tile([P, SEQ], F32, tag="pos", name=f"{nm0}_pos{mc}")
                    nc.sync.dma_start(out=pos_t, in_=post_d[mc])
                    hm = ph.tile([P, SEQ], F32, tag=f"h{s}", name=f"{nm0}_h{mc}")
                    nc.vector.tensor_add(hm, ps, pos_t)
                    ss_next.add(hm, f"{nm0}emb")
                    h.append(hm)
                    if mc % 4 == 3:
                        yield

                for li in range(N_LAYER):
                    nm = f"s{s}l{li}"
                    bo_sb = pbias.tile([P, DC], F32, tag="bo", name=f"{nm}_bo")
                    nc.sync.dma_start(out=bo_sb, in_=bo_d[li])
                    b1_sb = pbias.tile([P, FC], F32, tag="b1", name=f"{nm}_b1")
                    nc.sync.dma_start(out=b1_sb, in_=b1_d[li])
                    b2_sb = pbias.tile([P, DC], F32, tag="b2", name=f"{nm}_b2")
                    nc.sync.dma_start(out=b2_sb, in_=b2_d[li])

                    xn = norm_fin(h, ss_next, nm + "n1")
                    yield

                    # ---- V (token-major), needed before any AV ----
                    v_tiles = [pv.tile([P, HIDDEN], BF16, tag="v", name=f"{nm}_v{mc}")
                               for mc in range(TC)]
                    for nh in range(2):
                        wv_t = []
                        for kc in range(DC):
                            wvt = pw.tile([P, 512], BF16, tag="wv", bufs=9,
                                          name=f"{nm}_wv{nh}_{kc}")
                            nc.sync.dma_start(
                                out=wvt, in_=wv_d[li, kc, :, nh * 512:(nh + 1) * 512])
                            wv_t.append(wvt)
                        for mc in range(TC):
                            ps = pp_mm.tile([P, 512], F32, tag="mm",
                                            name=f"{nm}_v{nh}{mc}")
                            for kc in range(DC):
                                nc.tensor.matmul(ps, xn[kc][:, mc * P:(mc + 1) * P],
                                                 wv_t[kc],
                                                 start=(kc == 0), stop=(kc == DC - 1))
                            nc.scalar.copy(
                                out=v_tiles[mc][:, nh * 512:(nh + 1) * 512], in_=ps)
                            yield

                    # ---- attention, one head pair at a time ----
                    att_tiles = []
                    for ti in range(DC):
                        # Q/K for this pair (d-major chunk ti == heads 2ti,2ti+1)
                        qkt = []
                        for t in range(2):
                            wt = pw.tile([P, DC, P], BF16, tag="w",
                                         name=f"{nm}_wqk{t}_{ti}")
                            nc.sync.dma_start(out=wt, in_=wqk_d[li, t, ti])
                            ps = pp_mm.tile([P, SEQ], F32, tag="mm",
                                            name=f"{nm}_qk{t}{ti}")
                            for kc in range(DC):
                                nc.tensor.matmul(ps, wt[:, kc, :], xn[kc],
                                                 start=(kc == 0), stop=(kc == DC - 1))
                            dt_ = pqk.tile([P, SEQ], BF16, tag="qk",
                                           name=f"{nm}_t{t}{ti}")
                            nc.scalar.copy(out=dt_, in_=ps)
                            qkt.append(dt_)
                        qt, kt = qkt
                        yield

                        # scores + exp; heads of the pair on PE row-groups 0/64
                        E = [[], []]
                        for mc in range(TC):
                            for hp in range(2):
                                po = hp * HEAD
                                ps_s = pp_mm.tile([P, SEQ], F32, tag="mm",
                                                  name=f"{nm}_s{ti}_{hp}{mc}")
                                nc.tensor.matmul(
                                    ps_s, kt[po:po + HEAD, mc * P:(mc + 1) * P],
                                    qt[po:po + HEAD, :], start=True, stop=True)
                                e = pE.tile([P, SEQ], BF16, tag="E",
                                            name=f"{nm}_e{ti}_{hp}{mc}")
                                nc.scalar.activation(out=e, in_=ps_s, func=Act.Exp,
                                                     scale=scale, bias=zero_col)
                                E[hp].append(e)
                            if mc == 1:
                                yield
                        yield

                        # sumexp broadcast for both heads into one psum bank
                        tmp = [[], []]
                        for hp in range(2):
                            t1 = ptmp.tile([P, SEQ], BF16, tag="et",
                                           name=f"{nm}_t1_{ti}{hp}")
                            nc.vector.tensor_add(t1, E[hp][0], E[hp][1])
                            t2 = ptmp.tile([P, SEQ], BF16, tag="et",
                                           name=f"{nm}_t2_{ti}{hp}")
                            nc.vector.tensor_add(t2, E[hp][2], E[hp][3])
                            tmp[hp] = [t1, t2]
                        ps_sb = pp_pair.tile([P, SEQ], F32, tag="pair",
                                             name=f"{nm}_sb{ti}")
                        for hp in range(2):
                            for j in range(2):
                                nc.tensor.matmul(
                                    ps_sb[hp * HEAD:(hp + 1) * HEAD, :],
                                    ones64_b, tmp[hp][j],
                                    start=(j == 0), stop=(j == 1),
                                    skip_group_check=True)
                        rb = prb.tile([P, SEQ], F32, tag="rb", name=f"{nm}_rb{ti}")
                        nc.vector.reciprocal_approx_fast(out=rb, in_=ps_sb)
                        yield

                        # AV for both heads (col-groups 0/64), then normalize
                        ps_a = pp_pair.tile([P, SEQ], F32, tag="pair",
                                            name=f"{nm}_a{ti}")
                        for hp in range(2):
                            hh = 2 * ti + hp
                            for kc in range(TC):
                                nc.tensor.matmul(
                                    ps_a[hp * HEAD:(hp + 1) * HEAD, :],
                                    v_tiles[kc][:, hh * HEAD:(hh + 1) * HEAD],
                                    E[hp][kc],
                                    start=(kc == 0), stop=(kc == TC - 1),
                                    skip_group_check=True)
                        at = patt.tile([P, SEQ], BF16, tag="att",
                                       name=f"{nm}_at{ti}")
                        nc.vector.tensor_mul(at, ps_a, rb)
                        att_tiles.append(at)
                        yield

                    # ---- Wo + residual ----
                    ss_mid = SumSq(nm + "mid")
                    h2 = []
                    for mc in range(DC):
                        wt = pw.tile([P, DC, P], BF16, tag="w", name=f"{nm}_wo{mc}")
                        nc.sync.dma_start(out=wt, in_=wo_d[li, mc])
                        ps = pp_mm.tile([P, SEQ], F32, tag="mm", name=f"{nm}_o{mc}")
                        for kc in range(DC):
                            nc.tensor.matmul(ps, wt[:, kc, :], att_tiles[kc],
                                             start=(kc == 0), stop=(kc == DC - 1))
                        hn = ph.tile([P, SEQ], F32, tag=f"h{s}", name=f"{nm}_h2{mc}")
                        nc.vector.scalar_tensor_tensor(
                            out=hn, in0=ps, scalar=bo_sb[:, mc:mc + 1], in1=h[mc],
                            op0=Alu.add, op1=Alu.add)
                        ss_mid.add(hn, nm + "mid")
                        h2.append(hn)
                        if mc % 2 == 1:
                            yield
                    h = h2

                    # ---------------- FFN ----------------
                    yn = norm_fin(h, ss_mid, nm + "n2")
                    yield
                    g_tiles = []
                    for mc in range(FC):
                        wt = pw.tile([P, DC, P], BF16, tag="w", name=f"{nm}_w1{mc}")
                        nc.sync.dma_start(out=wt, in_=w1_d[li, mc])
                        ps = pp_mm.tile([P, SEQ], F32, tag="mm", name=f"{nm}_f1{mc}")
                        for kc in range(DC):
                            nc.tensor.matmul(ps, wt[:, kc, :], yn[kc],
                                             start=(kc == 0), stop=(kc == DC - 1))
                        g = pg.tile([P, SEQ], BF16, tag="g", name=f"{nm}_g{mc}")
                        nc.scalar.activation(out=g, in_=ps, func=Act.Gelu,
                                             bias=b1_sb[:, mc:mc + 1], scale=1.0)
                        g_tiles.append(g)
                        if mc % 2 == 1:
                            yield

                    if li < N_LAYER - 1:
                        ss_next = SumSq(nm + "nxt")
                    h3 = []
                    for mc in range(DC):
                        wt = pw.tile([P, FC, P], BF16, tag="w2", bufs=2,
                                     name=f"{nm}_w2{mc}")
                        nc.sync.dma_start(out=wt, in_=w2_d[li, mc])
                        ps = pp_mm.tile([P, SEQ], F32, tag="mm", name=f"{nm}_f2{mc}")
                        for kc in range(FC):
                            nc.tensor.matmul(ps, wt[:, kc, :], g_tiles[kc],
                                             start=(kc == 0), stop=(kc == FC - 1))
                        hn = ph.tile([P, SEQ], F32, tag=f"h{s}", name=f"{nm}_h3{mc}")
                        nc.vector.scalar_tensor_tensor(
                            out=hn, in0=ps, scalar=b2_sb[:, mc:mc + 1], in1=h[mc],
                            op0=Alu.add, op1=Alu.add)
                        if li < N_LAYER - 1:
                            ss_next.add(hn, nm + "nxt")
                        h3.append(hn)
                        yield
                    h = h3

                # ---------------- transpose + store ----------------
                for tck in range(TC):
                    ob = posb.tile([P, HIDDEN], F32, tag="osb", name=f"{nm0}_ob{tck}")
                    for dc in range(DC):
                        ps_t = pp_mm.tile([P, P], F32, tag="mm",
                                          name=f"{nm0}_tr{tck}_{dc}")
                        nc.tensor.transpose(ps_t, h[dc][:, tck * P:(tck + 1) * P],
                                            ident)
                        nc.vector.tensor_copy(out=ob[:, dc * P:(dc + 1) * P],
                                              in_=ps_t)
                    nc.sync.dma_start(out=out_d[s, tck * P:(tck + 1) * P, :], in_=ob)
                    yield

            gens = [seq_program(0), seq_program(1)]
            for _ in range(OFFSET):
                next(gens[0])
            alive = [True, True]
            while alive[0] or alive[1]:
                for i in range(2):
                    if alive[i]:
                        try:
                            next(gens[i])
                        except StopIteration:
                            alive[i] = False

    from concourse.library_overlay import lower_extended_insts
    lower_extended_insts(nc)   # populate .instr for custom-DVE InstISA ops
    if split_waits:
        _split_multiwait(nc)
    return nc


def _split_multiwait(nc, max_waits=1):
    """This container's walrus accepts at most one sync-wait per instruction;
    hoist excess waits onto standalone EventSemaphore ops on the same engine
    queue (queue order preserves semantics)."""
    import bass_rust
    from bass_rust import SyncInfo

    for fn in nc.m.functions:
        for blk in fn.blocks:
            out = []
            for inst in blk.instructions:
                si = inst.sync_info
                waits = list(si.on_wait) if si is not None and si.on_wait else []
                if len(waits) > max_waits:
                    extra, keep = waits[:-max_waits], waits[-max_waits:]
                    for i, w in enumerate(extra):
                        nop = bass_rust.InstEventSemaphore(
                            name=f"{inst.name}w{i}", engine=inst.engine)
                        nop.sync_info = SyncInfo(on_wait=[w], on_update=[])
                        out.append(nop)
                    inst.sync_info = SyncInfo(
                        on_wait=keep, on_update=list(si.on_update or []))
                out.append(inst)
            blk.instructions = out


def prep_inputs(inputs):
    """Host-side layout prep shared by all cores (weights identical per core)."""
    _ensure_paths()
    import ml_dtypes

    f32 = np.float32
    emb = np.asarray(inputs["emb_table"], f32)       # [32, 1023]
    pos = np.asarray(inputs["pos_table"], f32)       # [512, 1024]
    Wq = np.asarray(inputs["Wq"], f32)               # [6, 16, 1024, 64]
    Wk = np.asarray(inputs["Wk"], f32)
    Wv = np.asarray(inputs["Wv"], f32)
    Wo = np.asarray(inputs["Wo"], f32)               # [6, 1024, 1024]
    W1 = np.asarray(inputs["W1"], f32)               # [6, 1024, 2048]
    W2 = np.asarray(inputs["W2"], f32)               # [6, 2048, 1024]
    g1 = np.asarray(inputs["g1"], f32)               # [6, 1024]
    g2 = np.asarray(inputs["g2"], f32)

    wemb = np.zeros((VOCAB + 1, HIDDEN), ml_dtypes.bfloat16)
    wemb[:VOCAB, :HIDDEN - 1] = emb.astype(ml_dtypes.bfloat16)
    wemb[VOCAB, HIDDEN - 1] = 1.0                    # duration channel
    post = np.ascontiguousarray(pos.T.reshape(DC, P, SEQ))
    iota = np.arange(VOCAB, dtype=f32).reshape(VOCAB, 1)

    def blk_kxm(a, mchunks):
        # [K, M] -> [mc, p, kc, m] blocked for contiguous per-partition DMA
        k, m = a.shape
        return np.ascontiguousarray(
            a.reshape(k // P, P, mchunks, P).transpose(2, 1, 0, 3))

    bf16 = ml_dtypes.bfloat16
    wqk = np.empty((N_LAYER, 2, DC, P, DC, P), bf16)
    wv = np.empty((N_LAYER, DC, P, HIDDEN), bf16)
    wo = np.empty((N_LAYER, DC, P, DC, P), bf16)
    w1 = np.empty((N_LAYER, FC, P, DC, P), bf16)
    w2 = np.empty((N_LAYER, DC, P, FC, P), bf16)
    for i in range(N_LAYER):
        aq = (Wq[i] * g1[i][None, :, None]).transpose(1, 0, 2).reshape(HIDDEN, HIDDEN)
        ak = (Wk[i] * g1[i][None, :, None]).transpose(1, 0, 2).reshape(HIDDEN, HIDDEN)
        av = (Wv[i] * g1[i][None, :, None]).transpose(1, 0, 2).reshape(HIDDEN, HIDDEN)
        wqk[i, 0] = blk_kxm(aq, DC).astype(bf16)
        wqk[i, 1] = blk_kxm(ak, DC).astype(bf16)
        wv[i] = av.reshape(DC, P, HIDDEN).astype(bf16)
        wo[i] = blk_kxm(Wo[i], DC).astype(bf16)
        w1[i] = blk_kxm(g2[i][:, None] * W1[i], FC).astype(bf16)
        w2[i] = blk_kxm(W2[i], DC).astype(bf16)

    base = {
        "wemb": wemb, "post": post, "iota": iota,
        "wqk": wqk, "wv": wv, "wo": wo, "w1": w1, "w2": w2,
        "bo": np.ascontiguousarray(
            np.asarray(inputs["bo"], f32).reshape(N_LAYER, DC, P).transpose(0, 2, 1)),
        "b1": np.ascontiguousarray(
            np.asarray(inputs["b1"], f32).reshape(N_LAYER, FC, P).transpose(0, 2, 1)),
        "b2": np.ascontiguousarray(
            np.asarray(inputs["b2"], f32).reshape(N_LAYER, DC, P).transpose(0, 2, 1)),
    }
    return base


LAST_RESULTS = None


def _ntff_hook():
    """NTFF profiling hook via the axon .so (the concourse<->antenv bridge
    module is absent in this image, so drive the capture directly)."""
    try:
        from trn_agent_boot.trn_boot import _ntff_profile_via_ctypes
        return _ntff_profile_via_ctypes("/opt/axon/libaxon_pjrt.so")
    except Exception as e:
        print("ntff hook unavailable:", e)
        return None


def kernel(**inputs):
    global LAST_RESULTS
    _ensure_paths()
    from concourse.bass_utils import run_bass_kernel_spmd

    x = np.asarray(inputs["x"], np.float32)          # [16, 512, 2]
    base = prep_inputs(inputs)
    in_maps = []
    for c in range(N_CORES):
        m = dict(base)
        m["x"] = np.ascontiguousarray(x[c * SEQ_PER_CORE:(c + 1) * SEQ_PER_CORE])
        in_maps.append(m)

    nc = build_nc()
    trace_dir = os.environ.get("KBENCH_TRACE_DIR")
    if trace_dir:
        hook = _ntff_hook()
        if hook is not None:
            os.makedirs(trace_dir, exist_ok=True)
            with hook(trace_dir, [0]):
                res = run_bass_kernel_spmd(nc, in_maps, list(range(N_CORES)))
        else:
            res = run_bass_kernel_spmd(nc, in_maps, list(range(N_CORES)))
    else:
        res = run_bass_kernel_spmd(nc, in_maps, list(range(N_CORES)))
    LAST_RESULTS = res
    out = np.concatenate(
        [res.results[c]["out"].reshape(SEQ_PER_CORE, SEQ * HIDDEN)
         for c in range(N_CORES)], axis=0)
    return out


# revision 11
# speedup vs baseline: 1.4611x; 1.0209x over previous
"""Trainium2 Bass kernel for nn_AttentionEncoder (6-layer dense transformer).

Strategy (v2)
-------------
Data-parallel over batch: 16 sequences across 8 NeuronCores (2 per core), no
collectives.  Per core, each sequence's residual stream h lives in SBUF in
d-major layout ([HIDDEN, SEQ] as 8 tiles of [128, 512]); weights stream from
HBM in bf16; psum accumulation fp32.

v2 changes vs v1 (3.17ms):
  - The two sequences per core run as generator-interleaved instruction
    streams with a half-layer phase offset, so while one sequence is in its
    ACT-heavy attention phase the other feeds the PE dense FFN/QKV matmuls.
    This keeps TensorE busy (no 1-3us gaps) and therefore HAM-warm (2.4GHz
    instead of oscillating down to 1.2GHz, which alone was ~40% of v1 time).
  - Q/K are computed per head-pair right before that pair's attention
    (chunk mc of the d-major Q/K output == head pair mc), cutting q/k SBUF
    lifetime ~10x.
  - Softmax: per head-pair, sum+broadcast fused into ones[128,64]-stationary
    matmuls accumulating into one [128,512] psum bank (rows 0:64 = head h
    sumexp broadcast, 64:128 = head h'); one DVE reciprocal_approx_fast per
    pair (replaces v1's 3.3us serial [1,512] DVE reciprocal per head); one
    DVE multiply normalizes both heads into the d-major att tile.
  - Attention matmuls use base-partition slices so the scores of a head pair
    land on PE row-groups 0/64 and AV+bcast on col-groups 0/64, letting the
    16x 32x32 sub-array structure run both heads' matmuls concurrently.
  - RMSNorm: ACT Sqrt -> DVE reciprocal_approx_fast -> ones-row broadcast
    matmul (no serial DVE reciprocal).
  - Psum->SBUF evacuations moved to the Scalar engine (Copy needs no
    activation-table reload); DVE keeps the residual/normalize multiplies.
  - A post-pass splits multi-wait instructions into single-wait
    EventSemaphore prefixes (this container's walrus accepts one sync-wait
    per instruction).
"""

import os
import sys

import numpy as np

N_LAYER = 6
N_HEAD = 16
HIDDEN = 1024
HEAD = HIDDEN // N_HEAD
FFWD = 2048
SEQ = 512
VOCAB = 32
BATCH = 16
N_CORES = 8
SEQ_PER_CORE = BATCH // N_CORES

P = 128
DC = HIDDEN // P   # 8 d-chunks == 8 head pairs
FC = FFWD // P     # 16 f-chunks
TC = SEQ // P      # 4 token-chunks

OFFSET = 33        # units to prime seq 0 ahead of seq 1 (~half a layer)


def _ensure_paths():
    for p in (
        "/opt/trn_rl_repo",
        "/root/.axon_site",
        "/root/.axon_site/_ro/trn_rl_repo",
        "/root/.axon_site/_ro/pypackages",
    ):
        if os.path.isdir(p) and p not in sys.path:
            sys.path.append(p)


def build_nc(split_waits=True):
    _ensure_paths()
    import concourse.bass as bass
    import concourse.tile as tile
    from concourse import mybir
    from concourse.masks import make_identity

    F32 = mybir.dt.float32
    F32R = mybir.dt.float32r
    BF16 = mybir.dt.bfloat16
    Act = mybir.ActivationFunctionType
    Alu = mybir.AluOpType

    def r(ap):
        return ap.bitcast(F32R)

    nc = bass.Bass("TRN2", target_bir_lowering=False, debug=False)

    x_d = nc.dram_tensor("x", [SEQ_PER_CORE, SEQ, 2], F32, kind="ExternalInput").ap()
    wemb_d = nc.dram_tensor("wemb", [VOCAB + 1, HIDDEN], BF16, kind="ExternalInput").ap()
    post_d = nc.dram_tensor("post", [DC, P, SEQ], F32, kind="ExternalInput").ap()
    iota_d = nc.dram_tensor("iota", [VOCAB, 1], F32, kind="ExternalInput").ap()
    wqk_d = nc.dram_tensor("wqk", [N_LAYER, 2, DC, P, DC, P], BF16, kind="ExternalInput").ap()
    wv_d = nc.dram_tensor("wv", [N_LAYER, DC, P, HIDDEN], BF16, kind="ExternalInput").ap()
    wo_d = nc.dram_tensor("wo", [N_LAYER, DC, P, DC, P], BF16, kind="ExternalInput").ap()
    w1_d = nc.dram_tensor("w1", [N_LAYER, FC, P, DC, P], BF16, kind="ExternalInput").ap()
    w2_d = nc.dram_tensor("w2", [N_LAYER, DC, P, FC, P], BF16, kind="ExternalInput").ap()
    bo_d = nc.dram_tensor("bo", [N_LAYER, P, DC], F32, kind="ExternalInput").ap()
    b1_d = nc.dram_tensor("b1", [N_LAYER, P, FC], F32, kind="ExternalInput").ap()
    b2_d = nc.dram_tensor("b2", [N_LAYER, P, DC], F32, kind="ExternalInput").ap()
    out_d = nc.dram_tensor("out", [SEQ_PER_CORE, SEQ, HIDDEN], F32, kind="ExternalOutput").ap()

    eps = float(np.finfo(np.float32).eps)
    scale = float(HEAD ** -0.5)

    from contextlib import ExitStack

    with tile.TileContext(nc) as tc:
        with ExitStack() as ctx:
            pool = lambda *a, **kw: ctx.enter_context(tc.tile_pool(*a, **kw))
            pc = pool(name="pc", bufs=1)
            pst = pool(name="pst", bufs=3)
            ph = pool(name="ph", bufs=10)        # residual h, per-seq tag
            pxn = pool(name="pxn", bufs=18)      # xn/yn bf16
            pqk = pool(name="pqk", bufs=8)       # q/k pair tiles
            pv = pool(name="pv", bufs=8)         # v tiles [P, HIDDEN]
            pE = pool(name="pE", bufs=10)        # exp(scores)
            ptmp = pool(name="ptmp", bufs=5)     # E pair sums
            psq = pool(name="psq", bufs=4)       # h^2 for sumsq
            patt = pool(name="patt", bufs=9)    # attention output d-major
            pg = pool(name="pg", bufs=17)        # gelu outputs
            prb = pool(name="prb", bufs=2)       # softmax reciprocal bcast
            pw = pool(name="pw", bufs=5)         # weight chunks
            pbias = pool(name="pbias", bufs=3)
            ppos = pool(name="ppos", bufs=2)
            posb = pool(name="posb", bufs=2)
            pp_mm = pool(name="pp_mm", bufs=4, space="PSUM")
            pp_pair = pool(name="pp_pair", bufs=2, space="PSUM")
            pp_red = pool(name="pp_red", bufs=2, space="PSUM")

            # constants (memset cannot write fp32r; stage via f32 + copy)
            ones_f = pc.tile([P, P], F32, name="ones_f")
            nc.vector.memset(ones_f, 1.0)
            ones_row = pc.tile([1, P], F32R, name="ones_row")
            nc.vector.tensor_copy(out=ones_row, in_=ones_f[0:1, :])
            ones_col = pc.tile([P, 1], F32R, name="ones_col")
            nc.vector.tensor_copy(out=ones_col, in_=ones_f[:, 0:1])
            ones64_b = pc.tile([P, HEAD], BF16, name="ones64_b")
            nc.vector.tensor_copy(out=ones64_b, in_=ones_f[:, 0:HEAD])
            ones_row_b = pc.tile([1, P], BF16, name="ones_row_b")
            nc.vector.tensor_copy(out=ones_row_b, in_=ones_f[0:1, :])
            ident = pc.tile([P, P], F32, name="ident")
            make_identity(nc, ident)
            iota_t = pc.tile([VOCAB, 1], F32, name="iota_t")
            nc.sync.dma_start(out=iota_t, in_=iota_d)
            eps_t = pc.tile([1, 1], F32, name="eps_t")
            nc.vector.memset(eps_t, eps)
            zero_col = pc.tile([P, 1], F32, name="zero_col")
            nc.vector.memset(zero_col, 0.0)
            wemb_sb = pc.tile([VOCAB + 1, HIDDEN], BF16, name="wemb_sb")
            nc.sync.dma_start(out=wemb_sb, in_=wemb_d)

            class SumSq:
                """Accumulate sum over d of h^2 into a [1,SEQ] psum row."""

                def __init__(self, nm):
                    self.ps = pp_red.tile([1, SEQ], F32, tag="red", name=f"{nm}_ss")
                    self.started = False
                    self.n = 0

                def add(self, t, nm, total=DC):
                    sq = psq.tile([P, SEQ], F32R, tag="sq", name=f"{nm}_sq{self.n}")
                    nc.vector.tensor_mul(sq, t, t)
                    self.n += 1
                    nc.tensor.matmul(self.ps, r(ones_col), r(sq),
                                     start=not self.started,
                                     stop=(self.n == total))
                    self.started = True

            def norm_fin(h_tiles, ss, nm):
                rms = pst.tile([1, SEQ], F32, tag="stat", name=f"{nm}_rms")
                nc.scalar.activation(out=rms, in_=ss.ps, func=Act.Sqrt,
                                     scale=1.0 / HIDDEN, bias=eps_t)
                inv = pst.tile([1, SEQ], F32, tag="stat", name=f"{nm}_inv")
                nc.vector.reciprocal_approx_fast(out=inv, in_=rms)
                inv_r = pst.tile([1, SEQ], F32R, tag="stat", name=f"{nm}_invr")
                nc.vector.tensor_copy(out=inv_r, in_=inv)
                ps_b = pp_mm.tile([P, SEQ], F32, tag="mm", name=f"{nm}_bc")
                nc.tensor.matmul(ps_b, r(ones_row), inv_r, start=True, stop=True)
                xn = []
                for kc in range(DC):
                    xt = pxn.tile([P, SEQ], BF16, tag="xn", name=f"{nm}_xn{kc}")
                    nc.vector.tensor_mul(xt, h_tiles[kc], ps_b)
                    xn.append(xt)
                return xn

            def seq_program(s):
                nm0 = f"s{s}"
                # ---------------- embedding ----------------
                acts_f = pst.tile([1, SEQ], F32, tag="row", name=f"{nm0}_actsf")
                nc.sync.dma_start(out=acts_f, in_=x_d[s:s + 1, :, 0])
                acts = pst.tile([1, SEQ], BF16, tag="row", name=f"{nm0}_acts")
                nc.vector.tensor_copy(out=acts, in_=acts_f)
                dur = pst.tile([1, SEQ], F32, tag="row", name=f"{nm0}_dur")
                nc.sync.dma_start(out=dur, in_=x_d[s:s + 1, :, 1])
                ps_ab = pp_mm.tile([VOCAB, SEQ], F32, tag="mm", name=f"{nm0}_ab")
                nc.tensor.matmul(ps_ab, ones_row_b[:, :VOCAB], acts,
                                 start=True, stop=True)
                oh = pst.tile([VOCAB + 1, SEQ], BF16, tag="oh", bufs=2, name=f"{nm0}_oh")
                nc.vector.tensor_scalar(out=oh[0:VOCAB, :], in0=ps_ab,
                                        scalar1=iota_t, scalar2=None,
                                        op0=Alu.is_equal)
                nc.vector.tensor_copy(out=oh[VOCAB:VOCAB + 1, :], in_=dur)
                yield

                h = []
                ss_next = SumSq(f"{nm0}emb")
                for mc in range(DC):
                    ps = pp_mm.tile([P, SEQ], F32, tag="mm", name=f"{nm0}_emb{mc}")
                    nc.tensor.matmul(ps, wemb_sb[:, mc * P:(mc + 1) * P], oh,
                                     start=True, stop=True)
                    pos_t = ppos.tile([P, SEQ], F32, tag="pos", name=f"{nm0}_pos{mc}")
                    nc.sync.dma_start(out=pos_t, in_=post_d[mc])
                    hm = ph.tile([P, SEQ], F32, tag=f"h{s}", name=f"{nm0}_h{mc}")
                    nc.vector.tensor_add(hm, ps, pos_t)
                    ss_next.add(hm, f"{nm0}emb")
                    h.append(hm)
                    if mc % 4 == 3:
                        yield

                for li in range(N_LAYER):
                    nm = f"s{s}l{li}"
                    bo_sb = pbias.tile([P, DC], F32, tag="bo", name=f"{nm}_bo")
                    nc.sync.dma_start(out=bo_sb, in_=bo_d[li])
                    b1_sb = pbias.tile([P, FC], F32, tag="b1", name=f"{nm}_b1")
                    nc.sync.dma_start(out=b1_sb, in_=b1_d[li])
                    b2_sb = pbias.tile([P, DC], F32, tag="b2", name=f"{nm}_b2")
                    nc.sync.dma_start(out=b2_sb, in_=b2_d[li])

                    xn = norm_fin(h, ss_next, nm + "n1")
                    yield

                    # ---- V (token-major), needed before any AV ----
                    v_tiles = [pv.tile([P, HIDDEN], BF16, tag="v", name=f"{nm}_v{mc}")
                               for mc in range(TC)]
                    for nh in range(2):
                        wv_t = []
                        for kc in range(DC):
                            wvt = pw.tile([P, 512], BF16, tag="wv", bufs=9,
                                          name=f"{nm}_wv{nh}_{kc}")
                            nc.sync.dma_start(
                                out=wvt, in_=wv_d[li, kc, :, nh * 512:(nh + 1) * 512])
                            wv_t.append(wvt)
                        for mc in range(TC):
                            ps = pp_mm.tile([P, 512], F32, tag="mm",
                                            name=f"{nm}_v{nh}{mc}")
                            for kc in range(DC):
                                nc.tensor.matmul(ps, xn[kc][:, mc * P:(mc + 1) * P],
                                                 wv_t[kc],
                                                 start=(kc == 0), stop=(kc == DC - 1))
                            nc.scalar.copy(
                                out=v_tiles[mc][:, nh * 512:(nh + 1) * 512], in_=ps)
                            yield

                    # ---- attention, one head pair at a time ----
                    att_tiles = []
                    for ti in range(DC):
                        # Q/K for this pair (d-major chunk ti == heads 2ti,2ti+1)
                        qkt = []
                        for t in range(2):
                            wt = pw.tile([P, DC, P], BF16, tag="w",
                                         name=f"{nm}_wqk{t}_{ti}")
                            nc.sync.dma_start(out=wt, in_=wqk_d[li, t, ti])
                            ps = pp_mm.tile([P, SEQ], F32, tag="mm",
                                            name=f"{nm}_qk{t}{ti}")
                            for kc in range(DC):
                                nc.tensor.matmul(ps, wt[:, kc, :], xn[kc],
                                                 start=(kc == 0), stop=(kc == DC - 1))
                            dt_ = pqk.tile([P, SEQ], BF16, tag="qk",
                                           name=f"{nm}_t{t}{ti}")
                            nc.scalar.copy(out=dt_, in_=ps)
                            qkt.append(dt_)
                        qt, kt = qkt
                        yield

                        # scores + exp; heads of the pair on PE row-groups 0/64
                        E = [[], []]
                        for mc in range(TC):
                            for hp in range(2):
                                po = hp * HEAD
                                ps_s = pp_mm.tile([P, SEQ], F32, tag="mm",
                                                  name=f"{nm}_s{ti}_{hp}{mc}")
                                nc.tensor.matmul(
                                    ps_s, kt[po:po + HEAD, mc * P:(mc + 1) * P],
                                    qt[po:po + HEAD, :], start=True, stop=True)
                                e = pE.tile([P, SEQ], BF16, tag="E",
                                            name=f"{nm}_e{ti}_{hp}{mc}")
                                nc.scalar.activation(out=e, in_=ps_s, func=Act.Exp,
                                                     scale=scale, bias=zero_col)
                                E[hp].append(e)
                            if mc == 1:
                                yield
                        yield

                        # sumexp broadcast for both heads into one psum bank
                        esum = []
                        for hp in range(2):
                            t1 = ptmp.tile([P, SEQ], BF16, tag="et",
                                           name=f"{nm}_t1_{ti}{hp}")
                            nc.vector.tensor_add(t1, E[hp][0], E[hp][1])
                            t2 = ptmp.tile([P, SEQ], BF16, tag="et",
                                           name=f"{nm}_t2_{ti}{hp}")
                            nc.vector.tensor_add(t2, E[hp][2], E[hp][3])
                            t3 = ptmp.tile([P, SEQ], BF16, tag="et",
                                           name=f"{nm}_t3_{ti}{hp}")
                            nc.vector.tensor_add(t3, t1, t2)
                            esum.append(t3)
                        ps_sb = pp_mm.tile([P, SEQ], F32, tag="mm",
                                           name=f"{nm}_sb{ti}")
                        for hp in range(2):
                            nc.tensor.matmul(
                                ps_sb[hp * HEAD:(hp + 1) * HEAD, :],
                                ones64_b, esum[hp],
                                start=True, stop=True, skip_group_check=True)
                        rb = prb.tile([P, SEQ], F32, tag="rb", name=f"{nm}_rb{ti}")
                        nc.vector.reciprocal_approx_fast(out=rb, in_=ps_sb)
                        yield

                        # AV: the pair's heads in two banks on col-groups 0/64,
                        # interleaved so the 32x32 sub-arrays run them together
                        ps_av = [pp_pair.tile([P, SEQ], F32, tag="pair",
                                              name=f"{nm}_a{ti}{hp}")
                                 for hp in range(2)]
                        for kc in range(TC):
                            for hp in range(2):
                                hh = 2 * ti + hp
                                nc.tensor.matmul(
                                    ps_av[hp][hp * HEAD:(hp + 1) * HEAD, :],
                                    v_tiles[kc][:, hh * HEAD:(hh + 1) * HEAD],
                                    E[hp][kc],
                                    start=(kc == 0), stop=(kc == TC - 1),
                                    skip_group_check=True)
                        at = patt.tile([P, SEQ], BF16, tag="att",
                                       name=f"{nm}_at{ti}")
                        for hp in range(2):
                            sl = slice(hp * HEAD, (hp + 1) * HEAD)
                            nc.vector.tensor_mul(at[sl, :], ps_av[hp][sl, :],
                                                 rb[sl, :])
                        att_tiles.append(at)
                        yield

                    # ---- Wo + residual ----
                    ss_mid = SumSq(nm + "mid")
                    h2 = []
                    for mc in range(DC):
                        wt = pw.tile([P, DC, P], BF16, tag="w", name=f"{nm}_wo{mc}")
                        nc.sync.dma_start(out=wt, in_=wo_d[li, mc])
                        ps = pp_mm.tile([P, SEQ], F32, tag="mm", name=f"{nm}_o{mc}")
                        for kc in range(DC):
                            nc.tensor.matmul(ps, wt[:, kc, :], att_tiles[kc],
                                             start=(kc == 0), stop=(kc == DC - 1))
                        hn = ph.tile([P, SEQ], F32, tag=f"h{s}", name=f"{nm}_h2{mc}")
                        nc.vector.scalar_tensor_tensor(
                            out=hn, in0=ps, scalar=bo_sb[:, mc:mc + 1], in1=h[mc],
                            op0=Alu.add, op1=Alu.add)
                        ss_mid.add(hn, nm + "mid")
                        h2.append(hn)
                        if mc % 2 == 1:
                            yield
                    h = h2

                    # ---------------- FFN ----------------
                    yn = norm_fin(h, ss_mid, nm + "n2")
                    yield
                    g_tiles = []
                    for mc in range(FC):
                        wt = pw.tile([P, DC, P], BF16, tag="w", name=f"{nm}_w1{mc}")
                        nc.sync.dma_start(out=wt, in_=w1_d[li, mc])
                        ps = pp_mm.tile([P, SEQ], F32, tag="mm", name=f"{nm}_f1{mc}")
                        for kc in range(DC):
                            nc.tensor.matmul(ps, wt[:, kc, :], yn[kc],
                                             start=(kc == 0), stop=(kc == DC - 1))
                        g = pg.tile([P, SEQ], BF16, tag="g", name=f"{nm}_g{mc}")
                        nc.scalar.activation(out=g, in_=ps, func=Act.Gelu,
                                             bias=b1_sb[:, mc:mc + 1], scale=1.0)
                        g_tiles.append(g)
                        if mc % 4 == 3:
                            yield

                    if li < N_LAYER - 1:
                        ss_next = SumSq(nm + "nxt")
                    h3 = []
                    for mc in range(DC):
                        wt = pw.tile([P, FC, P], BF16, tag="w2", bufs=2,
                                     name=f"{nm}_w2{mc}")
                        nc.sync.dma_start(out=wt, in_=w2_d[li, mc])
                        ps = pp_mm.tile([P, SEQ], F32, tag="mm", name=f"{nm}_f2{mc}")
                        for kc in range(FC):
                            nc.tensor.matmul(ps, wt[:, kc, :], g_tiles[kc],
                                             start=(kc == 0), stop=(kc == FC - 1))
                        hn = ph.tile([P, SEQ], F32, tag=f"h{s}", name=f"{nm}_h3{mc}")
                        nc.vector.scalar_tensor_tensor(
                            out=hn, in0=ps, scalar=b2_sb[:, mc:mc + 1], in1=h[mc],
                            op0=Alu.add, op1=Alu.add)
                        if li < N_LAYER - 1:
                            ss_next.add(hn, nm + "nxt")
                        h3.append(hn)
                        yield
                    h = h3

                # ---------------- transpose + store ----------------
                for tck in range(TC):
                    ob = posb.tile([P, HIDDEN], F32, tag="osb", name=f"{nm0}_ob{tck}")
                    for dc in range(DC):
                        ps_t = pp_mm.tile([P, P], F32, tag="mm",
                                          name=f"{nm0}_tr{tck}_{dc}")
                        nc.tensor.transpose(ps_t, h[dc][:, tck * P:(tck + 1) * P],
                                            ident)
                        nc.vector.tensor_copy(out=ob[:, dc * P:(dc + 1) * P],
                                              in_=ps_t)
                    nc.sync.dma_start(out=out_d[s, tck * P:(tck + 1) * P, :], in_=ob)
                    yield

            gens = [seq_program(0), seq_program(1)]
            for _ in range(3):           # both embeddings first (PE density)
                next(gens[0])
                next(gens[1])
            for _ in range(OFFSET):
                next(gens[0])
            alive = [True, True]
            while alive[0] or alive[1]:
                for i in range(2):
                    if alive[i]:
                        try:
                            next(gens[i])
                        except StopIteration:
                            alive[i] = False

    from concourse.library_overlay import lower_extended_insts
    lower_extended_insts(nc)   # populate .instr for custom-DVE InstISA ops
    if split_waits:
        _split_multiwait(nc)
    return nc


def _split_multiwait(nc, max_waits=1):
    """This container's walrus accepts at most one sync-wait per instruction;
    hoist excess waits onto standalone EventSemaphore ops on the same engine
    queue (queue order preserves semantics)."""
    import bass_rust
    from bass_rust import SyncInfo

    for fn in nc.m.functions:
        for blk in fn.blocks:
            out = []
            for inst in blk.instructions:
                si = inst.sync_info
                waits = list(si.on_wait) if si is not None and si.on_wait else []
                if len(waits) > max_waits:
                    extra, keep = waits[:-max_waits], waits[-max_waits:]
                    for i, w in enumerate(extra):
                        nop = bass_rust.InstEventSemaphore(
                            name=f"{inst.name}w{i}", engine=inst.engine)
                        nop.sync_info = SyncInfo(on_wait=[w], on_update=[])
                        out.append(nop)
                    inst.sync_info = SyncInfo(
                        on_wait=keep, on_update=list(si.on_update or []))
                out.append(inst)
            blk.instructions = out


def prep_inputs(inputs):
    """Host-side layout prep shared by all cores (weights identical per core)."""
    _ensure_paths()
    import ml_dtypes

    f32 = np.float32
    emb = np.asarray(inputs["emb_table"], f32)       # [32, 1023]
    pos = np.asarray(inputs["pos_table"], f32)       # [512, 1024]
    Wq = np.asarray(inputs["Wq"], f32)               # [6, 16, 1024, 64]
    Wk = np.asarray(inputs["Wk"], f32)
    Wv = np.asarray(inputs["Wv"], f32)
    Wo = np.asarray(inputs["Wo"], f32)               # [6, 1024, 1024]
    W1 = np.asarray(inputs["W1"], f32)               # [6, 1024, 2048]
    W2 = np.asarray(inputs["W2"], f32)               # [6, 2048, 1024]
    g1 = np.asarray(inputs["g1"], f32)               # [6, 1024]
    g2 = np.asarray(inputs["g2"], f32)

    wemb = np.zeros((VOCAB + 1, HIDDEN), ml_dtypes.bfloat16)
    wemb[:VOCAB, :HIDDEN - 1] = emb.astype(ml_dtypes.bfloat16)
    wemb[VOCAB, HIDDEN - 1] = 1.0                    # duration channel
    post = np.ascontiguousarray(pos.T.reshape(DC, P, SEQ))
    iota = np.arange(VOCAB, dtype=f32).reshape(VOCAB, 1)

    def blk_kxm(a, mchunks):
        # [K, M] -> [mc, p, kc, m] blocked for contiguous per-partition DMA
        k, m = a.shape
        return np.ascontiguousarray(
            a.reshape(k // P, P, mchunks, P).transpose(2, 1, 0, 3))

    bf16 = ml_dtypes.bfloat16
    wqk = np.empty((N_LAYER, 2, DC, P, DC, P), bf16)
    wv = np.empty((N_LAYER, DC, P, HIDDEN), bf16)
    wo = np.empty((N_LAYER, DC, P, DC, P), bf16)
    w1 = np.empty((N_LAYER, FC, P, DC, P), bf16)
    w2 = np.empty((N_LAYER, DC, P, FC, P), bf16)
    for i in range(N_LAYER):
        aq = (Wq[i] * g1[i][None, :, None]).transpose(1, 0, 2).reshape(HIDDEN, HIDDEN)
        ak = (Wk[i] * g1[i][None, :, None]).transpose(1, 0, 2).reshape(HIDDEN, HIDDEN)
        av = (Wv[i] * g1[i][None, :, None]).transpose(1, 0, 2).reshape(HIDDEN, HIDDEN)
        wqk[i, 0] = blk_kxm(aq, DC).astype(bf16)
        wqk[i, 1] = blk_kxm(ak, DC).astype(bf16)
        wv[i] = av.reshape(DC, P, HIDDEN).astype(bf16)
        wo[i] = blk_kxm(Wo[i], DC).astype(bf16)
        w1[i] = blk_kxm(g2[i][:, None] * W1[i], FC).astype(bf16)
        w2[i] = blk_kxm(W2[i], DC).astype(bf16)

    base = {
        "wemb": wemb, "post": post, "iota": iota,
        "wqk": wqk, "wv": wv, "wo": wo, "w1": w1, "w2": w2,
        "bo": np.ascontiguousarray(
            np.asarray(inputs["bo"], f32).reshape(N_LAYER, DC, P).transpose(0, 2, 1)),
        "b1": np.ascontiguousarray(
            np.asarray(inputs["b1"], f32).reshape(N_LAYER, FC, P).transpose(0, 2, 1)),
        "b2": np.ascontiguousarray(
            np.asarray(inputs["b2"], f32).reshape(N_LAYER, DC, P).transpose(0, 2, 1)),
    }
    return base


LAST_RESULTS = None


def _ntff_hook():
    """NTFF profiling hook via the axon .so (the concourse<->antenv bridge
    module is absent in this image, so drive the capture directly)."""
    try:
        from trn_agent_boot.trn_boot import _ntff_profile_via_ctypes
        return _ntff_profile_via_ctypes("/opt/axon/libaxon_pjrt.so")
    except Exception as e:
        print("ntff hook unavailable:", e)
        return None


def kernel(**inputs):
    global LAST_RESULTS
    _ensure_paths()
    from concourse.bass_utils import run_bass_kernel_spmd

    x = np.asarray(inputs["x"], np.float32)          # [16, 512, 2]
    base = prep_inputs(inputs)
    in_maps = []
    for c in range(N_CORES):
        m = dict(base)
        m["x"] = np.ascontiguousarray(x[c * SEQ_PER_CORE:(c + 1) * SEQ_PER_CORE])
        in_maps.append(m)

    nc = build_nc()
    trace_dir = os.environ.get("KBENCH_TRACE_DIR")
    if trace_dir:
        hook = _ntff_hook()
        if hook is not None:
            os.makedirs(trace_dir, exist_ok=True)
            with hook(trace_dir, [0]):
                res = run_bass_kernel_spmd(nc, in_maps, list(range(N_CORES)))
        else:
            res = run_bass_kernel_spmd(nc, in_maps, list(range(N_CORES)))
    else:
        res = run_bass_kernel_spmd(nc, in_maps, list(range(N_CORES)))
    LAST_RESULTS = res
    out = np.concatenate(
        [res.results[c]["out"].reshape(SEQ_PER_CORE, SEQ * HIDDEN)
         for c in range(N_CORES)], axis=0)
    return out


# revision 14
# speedup vs baseline: 1.5059x; 1.0306x over previous
"""Trainium2 Bass kernel for nn_AttentionEncoder (6-layer dense transformer).

Strategy (v2)
-------------
Data-parallel over batch: 16 sequences across 8 NeuronCores (2 per core), no
collectives.  Per core, each sequence's residual stream h lives in SBUF in
d-major layout ([HIDDEN, SEQ] as 8 tiles of [128, 512]); weights stream from
HBM in bf16; psum accumulation fp32.

v2 changes vs v1 (3.17ms):
  - The two sequences per core run as generator-interleaved instruction
    streams with a half-layer phase offset, so while one sequence is in its
    ACT-heavy attention phase the other feeds the PE dense FFN/QKV matmuls.
    This keeps TensorE busy (no 1-3us gaps) and therefore HAM-warm (2.4GHz
    instead of oscillating down to 1.2GHz, which alone was ~40% of v1 time).
  - Q/K are computed per head-pair right before that pair's attention
    (chunk mc of the d-major Q/K output == head pair mc), cutting q/k SBUF
    lifetime ~10x.
  - Softmax: per head-pair, sum+broadcast fused into ones[128,64]-stationary
    matmuls accumulating into one [128,512] psum bank (rows 0:64 = head h
    sumexp broadcast, 64:128 = head h'); one DVE reciprocal_approx_fast per
    pair (replaces v1's 3.3us serial [1,512] DVE reciprocal per head); one
    DVE multiply normalizes both heads into the d-major att tile.
  - Attention matmuls use base-partition slices so the scores of a head pair
    land on PE row-groups 0/64 and AV+bcast on col-groups 0/64, letting the
    16x 32x32 sub-array structure run both heads' matmuls concurrently.
  - RMSNorm: ACT Sqrt -> DVE reciprocal_approx_fast -> ones-row broadcast
    matmul (no serial DVE reciprocal).
  - Psum->SBUF evacuations moved to the Scalar engine (Copy needs no
    activation-table reload); DVE keeps the residual/normalize multiplies.
  - A post-pass splits multi-wait instructions into single-wait
    EventSemaphore prefixes (this container's walrus accepts one sync-wait
    per instruction).
"""

import os
import sys

import numpy as np

N_LAYER = 6
N_HEAD = 16
HIDDEN = 1024
HEAD = HIDDEN // N_HEAD
FFWD = 2048
SEQ = 512
VOCAB = 32
BATCH = 16
N_CORES = 8
SEQ_PER_CORE = BATCH // N_CORES

P = 128
DC = HIDDEN // P   # 8 d-chunks == 8 head pairs
FC = FFWD // P     # 16 f-chunks
TC = SEQ // P      # 4 token-chunks

OFFSET = 33        # units to prime seq 0 ahead of seq 1 (~half a layer)


def _ensure_paths():
    for p in (
        "/opt/trn_rl_repo",
        "/root/.axon_site",
        "/root/.axon_site/_ro/trn_rl_repo",
        "/root/.axon_site/_ro/pypackages",
    ):
        if os.path.isdir(p) and p not in sys.path:
            sys.path.append(p)


def build_nc(split_waits=True):
    _ensure_paths()
    import concourse.bass as bass
    import concourse.tile as tile
    from concourse import mybir
    from concourse.masks import make_identity

    F32 = mybir.dt.float32
    F32R = mybir.dt.float32r
    BF16 = mybir.dt.bfloat16
    Act = mybir.ActivationFunctionType
    Alu = mybir.AluOpType

    def r(ap):
        return ap.bitcast(F32R)

    nc = bass.Bass("TRN2", target_bir_lowering=False, debug=False)

    x_d = nc.dram_tensor("x", [SEQ_PER_CORE, SEQ, 2], F32, kind="ExternalInput").ap()
    wemb_d = nc.dram_tensor("wemb", [VOCAB + 1, HIDDEN], BF16, kind="ExternalInput").ap()
    post_d = nc.dram_tensor("post", [DC, P, SEQ], F32, kind="ExternalInput").ap()
    iota_d = nc.dram_tensor("iota", [VOCAB, 1], F32, kind="ExternalInput").ap()
    wqk_d = nc.dram_tensor("wqk", [N_LAYER, 2, DC, P, DC, P], BF16, kind="ExternalInput").ap()
    wv_d = nc.dram_tensor("wv", [N_LAYER, DC, P, HIDDEN], BF16, kind="ExternalInput").ap()
    wo_d = nc.dram_tensor("wo", [N_LAYER, DC, P, DC, P], BF16, kind="ExternalInput").ap()
    w1_d = nc.dram_tensor("w1", [N_LAYER, FC, P, DC, P], BF16, kind="ExternalInput").ap()
    w2_d = nc.dram_tensor("w2", [N_LAYER, DC, P, FC, P], BF16, kind="ExternalInput").ap()
    bo_d = nc.dram_tensor("bo", [N_LAYER, P, DC], F32, kind="ExternalInput").ap()
    b1_d = nc.dram_tensor("b1", [N_LAYER, P, FC], F32, kind="ExternalInput").ap()
    b2_d = nc.dram_tensor("b2", [N_LAYER, P, DC], F32, kind="ExternalInput").ap()
    out_d = nc.dram_tensor("out", [SEQ_PER_CORE, SEQ, HIDDEN], F32, kind="ExternalOutput").ap()

    eps = float(np.finfo(np.float32).eps)
    scale = float(HEAD ** -0.5)

    from contextlib import ExitStack

    with tile.TileContext(nc) as tc:
        with ExitStack() as ctx:
            pool = lambda *a, **kw: ctx.enter_context(tc.tile_pool(*a, **kw))
            pc = pool(name="pc", bufs=1)
            pst = pool(name="pst", bufs=3)
            ph = pool(name="ph", bufs=10)        # residual h, per-seq tag
            pxn = pool(name="pxn", bufs=18)      # xn/yn bf16
            pqk = pool(name="pqk", bufs=8)       # q/k pair tiles
            pv = pool(name="pv", bufs=8)         # v tiles [P, HIDDEN]
            pE = pool(name="pE", bufs=10)        # exp(scores)
            ptmp = pool(name="ptmp", bufs=6)     # E pair sums
            psq = pool(name="psq", bufs=4)       # h^2 for sumsq
            patt = pool(name="patt", bufs=12)    # attention output d-major
            pg = pool(name="pg", bufs=17)        # gelu outputs
            prb = pool(name="prb", bufs=2)       # softmax reciprocal bcast
            pw = pool(name="pw", bufs=5)         # weight chunks
            pbias = pool(name="pbias", bufs=3)
            ppos = pool(name="ppos", bufs=2)
            posb = pool(name="posb", bufs=2)
            pp_big = pool(name="pp_big", bufs=2, space="PSUM")
            pp_s = pool(name="pp_s", bufs=2, space="PSUM")
            pp_x = pool(name="pp_x", bufs=1, space="PSUM")
            pp_pair = pool(name="pp_pair", bufs=1, space="PSUM")
            pp_red = pool(name="pp_red", bufs=2, space="PSUM")

            # constants (memset cannot write fp32r; stage via f32 + copy)
            ones_f = pc.tile([P, P], F32, name="ones_f")
            nc.vector.memset(ones_f, 1.0)
            ones_row = pc.tile([1, P], F32R, name="ones_row")
            nc.vector.tensor_copy(out=ones_row, in_=ones_f[0:1, :])
            ones_col = pc.tile([P, 1], F32R, name="ones_col")
            nc.vector.tensor_copy(out=ones_col, in_=ones_f[:, 0:1])
            ones64_b = pc.tile([P, HEAD], BF16, name="ones64_b")
            nc.vector.tensor_copy(out=ones64_b, in_=ones_f[:, 0:HEAD])
            ones_row_b = pc.tile([1, P], BF16, name="ones_row_b")
            nc.vector.tensor_copy(out=ones_row_b, in_=ones_f[0:1, :])
            ident = pc.tile([P, P], F32, name="ident")
            make_identity(nc, ident)
            iota_t = pc.tile([VOCAB, 1], F32, name="iota_t")
            nc.sync.dma_start(out=iota_t, in_=iota_d)
            eps_t = pc.tile([1, 1], F32, name="eps_t")
            nc.vector.memset(eps_t, eps)
            zero_col = pc.tile([P, 1], F32, name="zero_col")
            nc.vector.memset(zero_col, 0.0)
            wemb_sb = pc.tile([VOCAB + 1, HIDDEN], BF16, name="wemb_sb")
            nc.sync.dma_start(out=wemb_sb, in_=wemb_d)

            class SumSq:
                """Accumulate sum over d of h^2 into a [1,SEQ] psum row."""

                def __init__(self, nm):
                    self.ps = pp_red.tile([1, SEQ], F32, tag="red", name=f"{nm}_ss")
                    self.started = False
                    self.n = 0

                def add(self, t, nm, total=DC):
                    sq = psq.tile([P, SEQ], F32R, tag="sq", name=f"{nm}_sq{self.n}")
                    nc.vector.tensor_mul(sq, t, t)
                    self.n += 1
                    nc.tensor.matmul(self.ps, r(ones_col), r(sq),
                                     start=not self.started,
                                     stop=(self.n == total))
                    self.started = True

            def norm_fin(h_tiles, ss, nm):
                rms = pst.tile([1, SEQ], F32, tag="stat", name=f"{nm}_rms")
                nc.scalar.activation(out=rms, in_=ss.ps, func=Act.Sqrt,
                                     scale=1.0 / HIDDEN, bias=eps_t)
                inv = pst.tile([1, SEQ], F32, tag="stat", name=f"{nm}_inv")
                nc.vector.reciprocal_approx_fast(out=inv, in_=rms)
                inv_r = pst.tile([1, SEQ], F32R, tag="stat", name=f"{nm}_invr")
                nc.vector.tensor_copy(out=inv_r, in_=inv)
                ps_b = pp_x.tile([P, SEQ], F32, tag="mmx", name=f"{nm}_bc")
                nc.tensor.matmul(ps_b, r(ones_row), inv_r, start=True, stop=True)
                xn = []
                for kc in range(DC):
                    xt = pxn.tile([P, SEQ], BF16, tag="xn", name=f"{nm}_xn{kc}")
                    nc.vector.tensor_mul(xt, h_tiles[kc], ps_b)
                    xn.append(xt)
                return xn

            def seq_program(s):
                nm0 = f"s{s}"
                # ---------------- embedding ----------------
                acts_f = pst.tile([1, SEQ], F32, tag="row", name=f"{nm0}_actsf")
                nc.sync.dma_start(out=acts_f, in_=x_d[s:s + 1, :, 0])
                acts = pst.tile([1, SEQ], BF16, tag="row", name=f"{nm0}_acts")
                nc.vector.tensor_copy(out=acts, in_=acts_f)
                dur = pst.tile([1, SEQ], F32, tag="row", name=f"{nm0}_dur")
                nc.sync.dma_start(out=dur, in_=x_d[s:s + 1, :, 1])
                ps_ab = pp_big.tile([VOCAB, SEQ], F32, tag="mmb", name=f"{nm0}_ab")
                nc.tensor.matmul(ps_ab, ones_row_b[:, :VOCAB], acts,
                                 start=True, stop=True)
                oh = pst.tile([VOCAB + 1, SEQ], BF16, tag="oh", bufs=2,
                              name=f"{nm0}_oh")
                nc.vector.tensor_scalar(out=oh[0:VOCAB, :], in0=ps_ab,
                                        scalar1=iota_t, scalar2=None,
                                        op0=Alu.is_equal)
                nc.vector.tensor_copy(out=oh[VOCAB:VOCAB + 1, :], in_=dur)
                yield

                h = []
                hb = []
                ss_next = SumSq(f"{nm0}emb")
                for mc in range(DC):
                    ps = pp_big.tile([P, SEQ], F32, tag="mmb", name=f"{nm0}_emb{mc}")
                    nc.tensor.matmul(ps, wemb_sb[:, mc * P:(mc + 1) * P], oh,
                                     start=True, stop=True)
                    pos_t = ppos.tile([P, SEQ], F32, tag="pos", name=f"{nm0}_pos{mc}")
                    nc.sync.dma_start(out=pos_t, in_=post_d[mc])
                    hm = ph.tile([P, SEQ], F32, tag=f"h{s}", name=f"{nm0}_h{mc}")
                    nc.vector.tensor_add(hm, ps, pos_t)
                    hbm = pxn.tile([P, SEQ], BF16, tag="xn", name=f"{nm0}_hb{mc}")
                    nc.vector.tensor_copy(out=hbm, in_=hm)
                    ss_next.add(hm, f"{nm0}emb")
                    h.append(hm)
                    hb.append(hbm)
                    if mc % 4 == 3:
                        yield

                for li in range(N_LAYER):
                    nm = f"s{s}l{li}"
                    bo_sb = pbias.tile([P, DC], F32, tag="bo", name=f"{nm}_bo")
                    nc.sync.dma_start(out=bo_sb, in_=bo_d[li])
                    b1_sb = pbias.tile([P, FC], F32, tag="b1", name=f"{nm}_b1")
                    nc.sync.dma_start(out=b1_sb, in_=b1_d[li])
                    b2_sb = pbias.tile([P, DC], F32, tag="b2", name=f"{nm}_b2")
                    nc.sync.dma_start(out=b2_sb, in_=b2_d[li])

                    # ---- n1 finalize: normalizers only (h/hb stay raw) ----
                    rms = pst.tile([1, SEQ], F32, tag="stat", name=f"{nm}_rms")
                    nc.scalar.activation(out=rms, in_=ss_next.ps, func=Act.Sqrt,
                                         scale=1.0 / HIDDEN, bias=eps_t)
                    inv = pst.tile([1, SEQ], F32, tag="stat", name=f"{nm}_inv")
                    nc.vector.reciprocal_approx_fast(out=inv, in_=rms)
                    inv_r = pst.tile([1, SEQ], F32R, tag="stat", name=f"{nm}_invr")
                    nc.vector.tensor_copy(out=inv_r, in_=inv)
                    ps_b = pp_x.tile([P, SEQ], F32, tag="mmx", name=f"{nm}_bc")
                    nc.tensor.matmul(ps_b, r(ones_row), inv_r, start=True, stop=True)
                    nb = prb.tile([P, SEQ], F32, tag="nb", bufs=3, name=f"{nm}_nb")
                    nc.scalar.copy(out=nb, in_=ps_b)
                    invc = []    # inv as columns (V evac scale)
                    invcs = []   # inv*HEAD**-0.5 (exp scale AP, k-normalizer)
                    for mc in range(TC):
                        ps_t = pp_x.tile([P, 1], F32, tag="mmx",
                                         name=f"{nm}_it{mc}")
                        nc.tensor.transpose(
                            ps_t, inv[0:1, mc * P:(mc + 1) * P], ident[0:1, 0:1])
                        ic = pst.tile([P, 1], F32, tag="invc", bufs=18,
                                      name=f"{nm}_ic{mc}")
                        nc.scalar.copy(out=ic, in_=ps_t)
                        ics = pst.tile([P, 1], F32, tag="invc", bufs=18,
                                       name=f"{nm}_ics{mc}")
                        nc.scalar.mul(out=ics, in_=ps_t, mul=scale)
                        invc.append(ic)
                        invcs.append(ics)
                    yield

                    # ---- V from raw hb; normalizer applied at evacuation ----
                    v_tiles = [pv.tile([P, HIDDEN], BF16, tag="v", name=f"{nm}_v{mc}")
                               for mc in range(TC)]
                    for nh in range(2):
                        wv_t = []
                        for kc in range(DC):
                            wvt = pw.tile([P, 512], BF16, tag="wv", bufs=9,
                                          name=f"{nm}_wv{nh}_{kc}")
                            nc.sync.dma_start(
                                out=wvt, in_=wv_d[li, kc, :, nh * 512:(nh + 1) * 512])
                            wv_t.append(wvt)
                        for mc in range(TC):
                            ps = pp_big.tile([P, 512], F32, tag="mmb",
                                            name=f"{nm}_v{nh}{mc}")
                            for kc in range(DC):
                                nc.tensor.matmul(ps, hb[kc][:, mc * P:(mc + 1) * P],
                                                 wv_t[kc],
                                                 start=(kc == 0), stop=(kc == DC - 1))
                            nc.scalar.activation(
                                out=v_tiles[mc][:, nh * 512:(nh + 1) * 512],
                                in_=ps, func=Act.Copy, scale=invc[mc])
                            yield

                    # ---- attention: software-pipelined head pairs ----
                    att_tiles = [None] * DC

                    def sumav(ti, E, nmp):
                        esum = []
                        for hp in range(2):
                            t1 = ptmp.tile([P, SEQ], BF16, tag="et",
                                           name=f"{nmp}_t1_{ti}{hp}")
                            nc.vector.tensor_add(t1, E[hp][0], E[hp][1])
                            t2 = ptmp.tile([P, SEQ], BF16, tag="et",
                                           name=f"{nmp}_t2_{ti}{hp}")
                            nc.vector.tensor_add(t2, E[hp][2], E[hp][3])
                            t3 = ptmp.tile([P, SEQ], BF16, tag="et",
                                           name=f"{nmp}_t3_{ti}{hp}")
                            nc.vector.tensor_add(t3, t1, t2)
                            esum.append(t3)
                        ps_sb = pp_x.tile([P, SEQ], F32, tag="mmx",
                                          name=f"{nmp}_sb{ti}")
                        for hp in range(2):
                            nc.tensor.matmul(
                                ps_sb[hp * HEAD:(hp + 1) * HEAD, :],
                                ones64_b, esum[hp],
                                start=True, stop=True, skip_group_check=True)
                        rb = prb.tile([P, SEQ], F32, tag="rb", name=f"{nmp}_rb{ti}")
                        nc.vector.reciprocal_approx_fast(out=rb, in_=ps_sb)
                        ps_a = pp_pair.tile([P, SEQ], F32, tag="pair",
                                             name=f"{nmp}_a{ti}")
                        for hp in range(2):
                            hh = 2 * ti + hp
                            for kc in range(TC):
                                nc.tensor.matmul(
                                    ps_a[hp * HEAD:(hp + 1) * HEAD, :],
                                    v_tiles[kc][:, hh * HEAD:(hh + 1) * HEAD],
                                    E[hp][kc],
                                    start=(kc == 0), stop=(kc == TC - 1),
                                    skip_group_check=True)
                        at = patt.tile([P, SEQ], BF16, tag="att",
                                       name=f"{nmp}_at{ti}")
                        nc.vector.tensor_mul(at, ps_a, rb)
                        att_tiles[ti] = at

                    prevE = None
                    for ti in range(DC):
                        # Q/K for this pair from raw hb
                        wtq = pw.tile([P, DC, P], BF16, tag="w",
                                      name=f"{nm}_wq{ti}")
                        nc.sync.dma_start(out=wtq, in_=wqk_d[li, 0, ti])
                        ps_q = pp_x.tile([P, SEQ], F32, tag="mmx",
                                         name=f"{nm}_q{ti}")
                        for kc in range(DC):
                            nc.tensor.matmul(ps_q, wtq[:, kc, :], hb[kc],
                                             start=(kc == 0), stop=(kc == DC - 1))
                        qt = pqk.tile([P, SEQ], BF16, tag="qk", name=f"{nm}_qt{ti}")
                        nc.vector.tensor_mul(qt, ps_q, nb)   # fold inv[q]
                        wtk = pw.tile([P, DC, P], BF16, tag="w",
                                      name=f"{nm}_wk{ti}")
                        nc.sync.dma_start(out=wtk, in_=wqk_d[li, 1, ti])
                        ps_k = pp_s.tile([P, SEQ], F32, tag="mms",
                                         name=f"{nm}_k{ti}")
                        for kc in range(DC):
                            nc.tensor.matmul(ps_k, wtk[:, kc, :], hb[kc],
                                             start=(kc == 0), stop=(kc == DC - 1))
                        kt = pqk.tile([P, SEQ], BF16, tag="qk", name=f"{nm}_kt{ti}")
                        nc.scalar.copy(out=kt, in_=ps_k)     # raw K; inv[k] in exp
                        yield

                        # scores + exp (inv[k]*scale applied via ACT scale AP)
                        E = [[], []]
                        for mc in range(TC):
                            for hp in range(2):
                                po = hp * HEAD
                                ps_s = pp_s.tile([P, SEQ], F32, tag="mms",
                                                 name=f"{nm}_s{ti}_{hp}{mc}")
                                nc.tensor.matmul(
                                    ps_s, kt[po:po + HEAD, mc * P:(mc + 1) * P],
                                    qt[po:po + HEAD, :], start=True, stop=True)
                                e = pE.tile([P, SEQ], BF16, tag="E",
                                            name=f"{nm}_e{ti}_{hp}{mc}")
                                nc.scalar.activation(out=e, in_=ps_s, func=Act.Exp,
                                                     scale=invcs[mc], bias=zero_col)
                                E[hp].append(e)
                            if mc == 1:
                                yield
                        yield
                        if prevE is not None:
                            sumav(prevE[0], prevE[1], nm)
                            yield
                        prevE = (ti, E)
                    sumav(prevE[0], prevE[1], nm)
                    yield

                    # ---- Wo + residual ----
                    ss_mid = SumSq(nm + "mid")
                    h2 = []
                    for mc in range(DC):
                        wt = pw.tile([P, DC, P], BF16, tag="w", name=f"{nm}_wo{mc}")
                        nc.sync.dma_start(out=wt, in_=wo_d[li, mc])
                        ps = pp_big.tile([P, SEQ], F32, tag="mmb", name=f"{nm}_o{mc}")
                        for kc in range(DC):
                            nc.tensor.matmul(ps, wt[:, kc, :], att_tiles[kc],
                                             start=(kc == 0), stop=(kc == DC - 1))
                        hn = ph.tile([P, SEQ], F32, tag=f"h{s}", name=f"{nm}_h2{mc}")
                        nc.vector.scalar_tensor_tensor(
                            out=hn, in0=ps, scalar=bo_sb[:, mc:mc + 1], in1=h[mc],
                            op0=Alu.add, op1=Alu.add)
                        ss_mid.add(hn, nm + "mid")
                        h2.append(hn)
                        if mc % 2 == 1:
                            yield
                    h = h2

                    # ---------------- FFN ----------------
                    yn = norm_fin(h, ss_mid, nm + "n2")
                    yield
                    g_tiles = []
                    for mc in range(FC):
                        wt = pw.tile([P, DC, P], BF16, tag="w", name=f"{nm}_w1{mc}")
                        nc.sync.dma_start(out=wt, in_=w1_d[li, mc])
                        ps = pp_big.tile([P, SEQ], F32, tag="mmb", name=f"{nm}_f1{mc}")
                        for kc in range(DC):
                            nc.tensor.matmul(ps, wt[:, kc, :], yn[kc],
                                             start=(kc == 0), stop=(kc == DC - 1))
                        g = pg.tile([P, SEQ], BF16, tag="g", name=f"{nm}_g{mc}")
                        nc.scalar.activation(out=g, in_=ps, func=Act.Gelu,
                                             bias=b1_sb[:, mc:mc + 1], scale=1.0)
                        g_tiles.append(g)
                        if mc % 4 == 3:
                            yield

                    last = li == N_LAYER - 1
                    if not last:
                        ss_next = SumSq(nm + "nxt")
                    h3 = []
                    hb = []
                    for mc in range(DC):
                        wt = pw.tile([P, FC, P], BF16, tag="w2", bufs=2,
                                     name=f"{nm}_w2{mc}")
                        nc.sync.dma_start(out=wt, in_=w2_d[li, mc])
                        ps = pp_big.tile([P, SEQ], F32, tag="mmb", name=f"{nm}_f2{mc}")
                        for kc in range(FC):
                            nc.tensor.matmul(ps, wt[:, kc, :], g_tiles[kc],
                                             start=(kc == 0), stop=(kc == FC - 1))
                        hn = ph.tile([P, SEQ], F32, tag=f"h{s}", name=f"{nm}_h3{mc}")
                        nc.vector.scalar_tensor_tensor(
                            out=hn, in0=ps, scalar=b2_sb[:, mc:mc + 1], in1=h[mc],
                            op0=Alu.add, op1=Alu.add)
                        if not last:
                            ss_next.add(hn, nm + "nxt")
                            hbm = pxn.tile([P, SEQ], BF16, tag="xn",
                                           name=f"{nm}_hb{mc}")
                            nc.vector.tensor_copy(out=hbm, in_=hn)
                            hb.append(hbm)
                        h3.append(hn)
                        yield
                    h = h3

                # ---------------- transpose + store ----------------
                for tck in range(TC):
                    ob = posb.tile([P, HIDDEN], F32, tag="osb", name=f"{nm0}_ob{tck}")
                    for dc in range(DC):
                        ps_t = pp_big.tile([P, P], F32, tag="mmb",
                                           name=f"{nm0}_tr{tck}_{dc}")
                        nc.tensor.transpose(ps_t, h[dc][:, tck * P:(tck + 1) * P],
                                            ident)
                        nc.vector.tensor_copy(out=ob[:, dc * P:(dc + 1) * P],
                                              in_=ps_t)
                    nc.sync.dma_start(out=out_d[s, tck * P:(tck + 1) * P, :], in_=ob)
                    yield

            gens = [seq_program(0), seq_program(1)]
            for _ in range(3):           # both embeddings first (PE density)
                next(gens[0])
                next(gens[1])
            for _ in range(OFFSET):
                next(gens[0])
            alive = [True, True]
            while alive[0] or alive[1]:
                for i in range(2):
                    if alive[i]:
                        try:
                            next(gens[i])
                        except StopIteration:
                            alive[i] = False

    from concourse.library_overlay import lower_extended_insts
    lower_extended_insts(nc)   # populate .instr for custom-DVE InstISA ops
    if split_waits:
        _split_multiwait(nc)
    return nc


def _split_multiwait(nc, max_waits=1):
    """This container's walrus accepts at most one sync-wait per instruction;
    hoist excess waits onto standalone EventSemaphore ops on the same engine
    queue (queue order preserves semantics)."""
    import bass_rust
    from bass_rust import SyncInfo

    for fn in nc.m.functions:
        for blk in fn.blocks:
            out = []
            for inst in blk.instructions:
                si = inst.sync_info
                waits = list(si.on_wait) if si is not None and si.on_wait else []
                if len(waits) > max_waits:
                    extra, keep = waits[:-max_waits], waits[-max_waits:]
                    for i, w in enumerate(extra):
                        nop = bass_rust.InstEventSemaphore(
                            name=f"{inst.name}w{i}", engine=inst.engine)
                        nop.sync_info = SyncInfo(on_wait=[w], on_update=[])
                        out.append(nop)
                    inst.sync_info = SyncInfo(
                        on_wait=keep, on_update=list(si.on_update or []))
                out.append(inst)
            blk.instructions = out


def prep_inputs(inputs):
    """Host-side layout prep shared by all cores (weights identical per core)."""
    _ensure_paths()
    import ml_dtypes

    f32 = np.float32
    emb = np.asarray(inputs["emb_table"], f32)       # [32, 1023]
    pos = np.asarray(inputs["pos_table"], f32)       # [512, 1024]
    Wq = np.asarray(inputs["Wq"], f32)               # [6, 16, 1024, 64]
    Wk = np.asarray(inputs["Wk"], f32)
    Wv = np.asarray(inputs["Wv"], f32)
    Wo = np.asarray(inputs["Wo"], f32)               # [6, 1024, 1024]
    W1 = np.asarray(inputs["W1"], f32)               # [6, 1024, 2048]
    W2 = np.asarray(inputs["W2"], f32)               # [6, 2048, 1024]
    g1 = np.asarray(inputs["g1"], f32)               # [6, 1024]
    g2 = np.asarray(inputs["g2"], f32)

    wemb = np.zeros((VOCAB + 1, HIDDEN), ml_dtypes.bfloat16)
    wemb[:VOCAB, :HIDDEN - 1] = emb.astype(ml_dtypes.bfloat16)
    wemb[VOCAB, HIDDEN - 1] = 1.0                    # duration channel
    post = np.ascontiguousarray(pos.T.reshape(DC, P, SEQ))
    iota = np.arange(VOCAB, dtype=f32).reshape(VOCAB, 1)

    def blk_kxm(a, mchunks):
        # [K, M] -> [mc, p, kc, m] blocked for contiguous per-partition DMA
        k, m = a.shape
        return np.ascontiguousarray(
            a.reshape(k // P, P, mchunks, P).transpose(2, 1, 0, 3))

    bf16 = ml_dtypes.bfloat16
    wqk = np.empty((N_LAYER, 2, DC, P, DC, P), bf16)
    wv = np.empty((N_LAYER, DC, P, HIDDEN), bf16)
    wo = np.empty((N_LAYER, DC, P, DC, P), bf16)
    w1 = np.empty((N_LAYER, FC, P, DC, P), bf16)
    w2 = np.empty((N_LAYER, DC, P, FC, P), bf16)
    for i in range(N_LAYER):
        aq = (Wq[i] * g1[i][None, :, None]).transpose(1, 0, 2).reshape(HIDDEN, HIDDEN)
        ak = (Wk[i] * g1[i][None, :, None]).transpose(1, 0, 2).reshape(HIDDEN, HIDDEN)
        av = (Wv[i] * g1[i][None, :, None]).transpose(1, 0, 2).reshape(HIDDEN, HIDDEN)
        wqk[i, 0] = blk_kxm(aq, DC).astype(bf16)
        wqk[i, 1] = blk_kxm(ak, DC).astype(bf16)
        wv[i] = av.reshape(DC, P, HIDDEN).astype(bf16)
        wo[i] = blk_kxm(Wo[i], DC).astype(bf16)
        w1[i] = blk_kxm(g2[i][:, None] * W1[i], FC).astype(bf16)
        w2[i] = blk_kxm(W2[i], DC).astype(bf16)

    base = {
        "wemb": wemb, "post": post, "iota": iota,
        "wqk": wqk, "wv": wv, "wo": wo, "w1": w1, "w2": w2,
        "bo": np.ascontiguousarray(
            np.asarray(inputs["bo"], f32).reshape(N_LAYER, DC, P).transpose(0, 2, 1)),
        "b1": np.ascontiguousarray(
            np.asarray(inputs["b1"], f32).reshape(N_LAYER, FC, P).transpose(0, 2, 1)),
        "b2": np.ascontiguousarray(
            np.asarray(inputs["b2"], f32).reshape(N_LAYER, DC, P).transpose(0, 2, 1)),
    }
    return base


LAST_RESULTS = None


def _ntff_hook():
    """NTFF profiling hook via the axon .so (the concourse<->antenv bridge
    module is absent in this image, so drive the capture directly)."""
    try:
        from trn_agent_boot.trn_boot import _ntff_profile_via_ctypes
        return _ntff_profile_via_ctypes("/opt/axon/libaxon_pjrt.so")
    except Exception as e:
        print("ntff hook unavailable:", e)
        return None


def kernel(**inputs):
    global LAST_RESULTS
    _ensure_paths()
    from concourse.bass_utils import run_bass_kernel_spmd

    x = np.asarray(inputs["x"], np.float32)          # [16, 512, 2]
    base = prep_inputs(inputs)
    in_maps = []
    for c in range(N_CORES):
        m = dict(base)
        m["x"] = np.ascontiguousarray(x[c * SEQ_PER_CORE:(c + 1) * SEQ_PER_CORE])
        in_maps.append(m)

    nc = build_nc()
    trace_dir = os.environ.get("KBENCH_TRACE_DIR")
    if trace_dir:
        hook = _ntff_hook()
        if hook is not None:
            os.makedirs(trace_dir, exist_ok=True)
            with hook(trace_dir, [0]):
                res = run_bass_kernel_spmd(nc, in_maps, list(range(N_CORES)))
        else:
            res = run_bass_kernel_spmd(nc, in_maps, list(range(N_CORES)))
    else:
        res = run_bass_kernel_spmd(nc, in_maps, list(range(N_CORES)))
    LAST_RESULTS = res
    out = np.concatenate(
        [res.results[c]["out"].reshape(SEQ_PER_CORE, SEQ * HIDDEN)
         for c in range(N_CORES)], axis=0)
    return out


# revision 15
# speedup vs baseline: 1.5172x; 1.0076x over previous
"""Trainium2 Bass kernel for nn_AttentionEncoder (6-layer dense transformer).

Strategy (v2)
-------------
Data-parallel over batch: 16 sequences across 8 NeuronCores (2 per core), no
collectives.  Per core, each sequence's residual stream h lives in SBUF in
d-major layout ([HIDDEN, SEQ] as 8 tiles of [128, 512]); weights stream from
HBM in bf16; psum accumulation fp32.

v2 changes vs v1 (3.17ms):
  - The two sequences per core run as generator-interleaved instruction
    streams with a half-layer phase offset, so while one sequence is in its
    ACT-heavy attention phase the other feeds the PE dense FFN/QKV matmuls.
    This keeps TensorE busy (no 1-3us gaps) and therefore HAM-warm (2.4GHz
    instead of oscillating down to 1.2GHz, which alone was ~40% of v1 time).
  - Q/K are computed per head-pair right before that pair's attention
    (chunk mc of the d-major Q/K output == head pair mc), cutting q/k SBUF
    lifetime ~10x.
  - Softmax: per head-pair, sum+broadcast fused into ones[128,64]-stationary
    matmuls accumulating into one [128,512] psum bank (rows 0:64 = head h
    sumexp broadcast, 64:128 = head h'); one DVE reciprocal_approx_fast per
    pair (replaces v1's 3.3us serial [1,512] DVE reciprocal per head); one
    DVE multiply normalizes both heads into the d-major att tile.
  - Attention matmuls use base-partition slices so the scores of a head pair
    land on PE row-groups 0/64 and AV+bcast on col-groups 0/64, letting the
    16x 32x32 sub-array structure run both heads' matmuls concurrently.
  - RMSNorm: ACT Sqrt -> DVE reciprocal_approx_fast -> ones-row broadcast
    matmul (no serial DVE reciprocal).
  - Psum->SBUF evacuations moved to the Scalar engine (Copy needs no
    activation-table reload); DVE keeps the residual/normalize multiplies.
  - A post-pass splits multi-wait instructions into single-wait
    EventSemaphore prefixes (this container's walrus accepts one sync-wait
    per instruction).
"""

import os
import sys

import numpy as np

N_LAYER = 6
N_HEAD = 16
HIDDEN = 1024
HEAD = HIDDEN // N_HEAD
FFWD = 2048
SEQ = 512
VOCAB = 32
BATCH = 16
N_CORES = 8
SEQ_PER_CORE = BATCH // N_CORES

P = 128
DC = HIDDEN // P   # 8 d-chunks == 8 head pairs
FC = FFWD // P     # 16 f-chunks
TC = SEQ // P      # 4 token-chunks

OFFSET = 33        # units to prime seq 0 ahead of seq 1 (~half a layer)


def _ensure_paths():
    for p in (
        "/opt/trn_rl_repo",
        "/root/.axon_site",
        "/root/.axon_site/_ro/trn_rl_repo",
        "/root/.axon_site/_ro/pypackages",
    ):
        if os.path.isdir(p) and p not in sys.path:
            sys.path.append(p)


def build_nc(split_waits=True):
    _ensure_paths()
    import concourse.bass as bass
    import concourse.tile as tile
    from concourse import mybir
    from concourse.masks import make_identity

    F32 = mybir.dt.float32
    F32R = mybir.dt.float32r
    BF16 = mybir.dt.bfloat16
    Act = mybir.ActivationFunctionType
    Alu = mybir.AluOpType

    def r(ap):
        return ap.bitcast(F32R)

    nc = bass.Bass("TRN2", target_bir_lowering=False, debug=False)

    x_d = nc.dram_tensor("x", [SEQ_PER_CORE, SEQ, 2], F32, kind="ExternalInput").ap()
    wemb_d = nc.dram_tensor("wemb", [VOCAB + 1, HIDDEN], BF16, kind="ExternalInput").ap()
    post_d = nc.dram_tensor("post", [DC, P, SEQ], F32, kind="ExternalInput").ap()
    iota_d = nc.dram_tensor("iota", [VOCAB, 1], F32, kind="ExternalInput").ap()
    wqk_d = nc.dram_tensor("wqk", [N_LAYER, 2, DC, P, DC, P], BF16, kind="ExternalInput").ap()
    wv_d = nc.dram_tensor("wv", [N_LAYER, DC, P, HIDDEN], BF16, kind="ExternalInput").ap()
    wo_d = nc.dram_tensor("wo", [N_LAYER, DC, P, DC, P], BF16, kind="ExternalInput").ap()
    w1_d = nc.dram_tensor("w1", [N_LAYER, FC, P, DC, P], BF16, kind="ExternalInput").ap()
    w2_d = nc.dram_tensor("w2", [N_LAYER, DC, P, FC, P], BF16, kind="ExternalInput").ap()
    bo_d = nc.dram_tensor("bo", [N_LAYER, P, DC], F32, kind="ExternalInput").ap()
    b1_d = nc.dram_tensor("b1", [N_LAYER, P, FC], F32, kind="ExternalInput").ap()
    b2_d = nc.dram_tensor("b2", [N_LAYER, P, DC], F32, kind="ExternalInput").ap()
    out_d = nc.dram_tensor("out", [SEQ_PER_CORE, SEQ, HIDDEN], F32, kind="ExternalOutput").ap()

    eps = float(np.finfo(np.float32).eps)
    scale = float(HEAD ** -0.5)

    from contextlib import ExitStack

    with tile.TileContext(nc) as tc:
        with ExitStack() as ctx:
            pool = lambda *a, **kw: ctx.enter_context(tc.tile_pool(*a, **kw))
            pc = pool(name="pc", bufs=1)
            pst = pool(name="pst", bufs=3)
            ph = pool(name="ph", bufs=10)        # residual h, per-seq tag
            pxn = pool(name="pxn", bufs=18)      # xn/yn bf16
            pqk = pool(name="pqk", bufs=8)       # q/k pair tiles
            pv = pool(name="pv", bufs=8)         # v tiles [P, HIDDEN]
            pE = pool(name="pE", bufs=10)        # exp(scores)
            ptmp = pool(name="ptmp", bufs=6)     # E pair sums
            psq = pool(name="psq", bufs=4)       # h^2 for sumsq
            patt = pool(name="patt", bufs=12)    # attention output d-major
            pg = pool(name="pg", bufs=17)        # gelu outputs
            prb = pool(name="prb", bufs=2)       # softmax reciprocal bcast
            pw = pool(name="pw", bufs=5)         # weight chunks
            pbias = pool(name="pbias", bufs=3)
            ppos = pool(name="ppos", bufs=2)
            posb = pool(name="posb", bufs=2)
            pp_big = pool(name="pp_big", bufs=2, space="PSUM")
            pp_s = pool(name="pp_s", bufs=2, space="PSUM")
            pp_x = pool(name="pp_x", bufs=1, space="PSUM")
            pp_pair = pool(name="pp_pair", bufs=1, space="PSUM")
            pp_red = pool(name="pp_red", bufs=2, space="PSUM")

            # constants (memset cannot write fp32r; stage via f32 + copy)
            ones_f = pc.tile([P, P], F32, name="ones_f")
            nc.vector.memset(ones_f, 1.0)
            ones_row = pc.tile([1, P], F32R, name="ones_row")
            nc.vector.tensor_copy(out=ones_row, in_=ones_f[0:1, :])
            ones_col = pc.tile([P, 1], F32R, name="ones_col")
            nc.vector.tensor_copy(out=ones_col, in_=ones_f[:, 0:1])
            ones64_b = pc.tile([P, HEAD], BF16, name="ones64_b")
            nc.vector.tensor_copy(out=ones64_b, in_=ones_f[:, 0:HEAD])
            ones_row_b = pc.tile([1, P], BF16, name="ones_row_b")
            nc.vector.tensor_copy(out=ones_row_b, in_=ones_f[0:1, :])
            ident = pc.tile([P, P], F32, name="ident")
            make_identity(nc, ident)
            iota_t = pc.tile([VOCAB, 1], F32, name="iota_t")
            nc.sync.dma_start(out=iota_t, in_=iota_d)
            eps_t = pc.tile([1, 1], F32, name="eps_t")
            nc.vector.memset(eps_t, eps)
            zero_col = pc.tile([P, 1], F32, name="zero_col")
            nc.vector.memset(zero_col, 0.0)
            wemb_sb = pc.tile([VOCAB + 1, HIDDEN], BF16, name="wemb_sb")
            nc.sync.dma_start(out=wemb_sb, in_=wemb_d)

            class SumSq:
                """Accumulate sum over d of h^2 into a [1,SEQ] psum row."""

                def __init__(self, nm):
                    self.ps = pp_red.tile([1, SEQ], F32, tag="red", name=f"{nm}_ss")
                    self.started = False
                    self.n = 0

                def add(self, t, nm, total=DC):
                    sq = psq.tile([P, SEQ], F32R, tag="sq", name=f"{nm}_sq{self.n}")
                    nc.vector.tensor_mul(sq, t, t)
                    self.n += 1
                    nc.tensor.matmul(self.ps, r(ones_col), r(sq),
                                     start=not self.started,
                                     stop=(self.n == total))
                    self.started = True

            def norm_fin(h_tiles, ss, nm):
                rms = pst.tile([1, SEQ], F32, tag="stat", name=f"{nm}_rms")
                nc.scalar.activation(out=rms, in_=ss.ps, func=Act.Sqrt,
                                     scale=1.0 / HIDDEN, bias=eps_t)
                inv = pst.tile([1, SEQ], F32, tag="stat", name=f"{nm}_inv")
                nc.vector.reciprocal_approx_fast(out=inv, in_=rms)
                inv_r = pst.tile([1, SEQ], F32R, tag="stat", name=f"{nm}_invr")
                nc.vector.tensor_copy(out=inv_r, in_=inv)
                ps_b = pp_x.tile([P, SEQ], F32, tag="mmx", name=f"{nm}_bc")
                nc.tensor.matmul(ps_b, r(ones_row), inv_r, start=True, stop=True)
                xn = []
                for kc in range(DC):
                    xt = pxn.tile([P, SEQ], BF16, tag="xn", name=f"{nm}_xn{kc}")
                    nc.vector.tensor_mul(xt, h_tiles[kc], ps_b)
                    xn.append(xt)
                return xn

            def seq_program(s):
                nm0 = f"s{s}"
                # ---------------- embedding ----------------
                acts_f = pst.tile([1, SEQ], F32, tag="row", name=f"{nm0}_actsf")
                nc.sync.dma_start(out=acts_f, in_=x_d[s:s + 1, :, 0])
                acts = pst.tile([1, SEQ], BF16, tag="row", name=f"{nm0}_acts")
                nc.vector.tensor_copy(out=acts, in_=acts_f)
                dur = pst.tile([1, SEQ], F32, tag="row", name=f"{nm0}_dur")
                nc.sync.dma_start(out=dur, in_=x_d[s:s + 1, :, 1])
                ps_ab = pp_big.tile([VOCAB, SEQ], F32, tag="mmb", name=f"{nm0}_ab")
                nc.tensor.matmul(ps_ab, ones_row_b[:, :VOCAB], acts,
                                 start=True, stop=True)
                oh = pst.tile([VOCAB + 1, SEQ], BF16, tag="oh", bufs=2,
                              name=f"{nm0}_oh")
                nc.vector.tensor_scalar(out=oh[0:VOCAB, :], in0=ps_ab,
                                        scalar1=iota_t, scalar2=None,
                                        op0=Alu.is_equal)
                nc.vector.tensor_copy(out=oh[VOCAB:VOCAB + 1, :], in_=dur)
                yield

                h = []
                hb = []
                ss_next = SumSq(f"{nm0}emb")
                for mc in range(DC):
                    ps = pp_big.tile([P, SEQ], F32, tag="mmb", name=f"{nm0}_emb{mc}")
                    nc.tensor.matmul(ps, wemb_sb[:, mc * P:(mc + 1) * P], oh,
                                     start=True, stop=True)
                    pos_t = ppos.tile([P, SEQ], F32, tag="pos", name=f"{nm0}_pos{mc}")
                    nc.sync.dma_start(out=pos_t, in_=post_d[mc])
                    hm = ph.tile([P, SEQ], F32, tag=f"h{s}", name=f"{nm0}_h{mc}")
                    nc.vector.tensor_add(hm, ps, pos_t)
                    hbm = pxn.tile([P, SEQ], BF16, tag="xn", name=f"{nm0}_hb{mc}")
                    nc.vector.tensor_copy(out=hbm, in_=hm)
                    ss_next.add(hm, f"{nm0}emb")
                    h.append(hm)
                    hb.append(hbm)
                    if mc % 4 == 3:
                        yield

                for li in range(N_LAYER):
                    nm = f"s{s}l{li}"
                    bo_sb = pbias.tile([P, DC], F32, tag="bo", name=f"{nm}_bo")
                    nc.sync.dma_start(out=bo_sb, in_=bo_d[li])
                    b1_sb = pbias.tile([P, FC], F32, tag="b1", name=f"{nm}_b1")
                    nc.sync.dma_start(out=b1_sb, in_=b1_d[li])
                    b2_sb = pbias.tile([P, DC], F32, tag="b2", name=f"{nm}_b2")
                    nc.sync.dma_start(out=b2_sb, in_=b2_d[li])

                    # ---- n1 finalize: normalizers only (h/hb stay raw) ----
                    rms = pst.tile([1, SEQ], F32, tag="stat", name=f"{nm}_rms")
                    nc.scalar.activation(out=rms, in_=ss_next.ps, func=Act.Sqrt,
                                         scale=1.0 / HIDDEN, bias=eps_t)
                    inv = pst.tile([1, SEQ], F32, tag="stat", name=f"{nm}_inv")
                    nc.vector.reciprocal_approx_fast(out=inv, in_=rms)
                    inv_r = pst.tile([1, SEQ], F32R, tag="stat", name=f"{nm}_invr")
                    nc.vector.tensor_copy(out=inv_r, in_=inv)
                    ps_b = pp_x.tile([P, SEQ], F32, tag="mmx", name=f"{nm}_bc")
                    nc.tensor.matmul(ps_b, r(ones_row), inv_r, start=True, stop=True)
                    nb = prb.tile([P, SEQ], F32, tag="nb", bufs=3, name=f"{nm}_nb")
                    nc.scalar.copy(out=nb, in_=ps_b)
                    invc = []    # inv as columns (V evac scale)
                    for mc in range(TC):
                        ps_t = pp_x.tile([P, 1], F32, tag="mmx",
                                         name=f"{nm}_it{mc}")
                        nc.tensor.transpose(
                            ps_t, inv[0:1, mc * P:(mc + 1) * P], ident[0:1, 0:1])
                        ic = pst.tile([P, 1], F32, tag="invc", bufs=10,
                                      name=f"{nm}_ic{mc}")
                        nc.scalar.copy(out=ic, in_=ps_t)
                        invc.append(ic)
                    yield

                    # ---- V from raw hb; normalizer applied at evacuation ----
                    v_tiles = [pv.tile([P, HIDDEN], BF16, tag="v", name=f"{nm}_v{mc}")
                               for mc in range(TC)]
                    for nh in range(2):
                        wv_t = []
                        for kc in range(DC):
                            wvt = pw.tile([P, 512], BF16, tag="wv", bufs=9,
                                          name=f"{nm}_wv{nh}_{kc}")
                            nc.sync.dma_start(
                                out=wvt, in_=wv_d[li, kc, :, nh * 512:(nh + 1) * 512])
                            wv_t.append(wvt)
                        for mc in range(TC):
                            ps = pp_big.tile([P, 512], F32, tag="mmb",
                                            name=f"{nm}_v{nh}{mc}")
                            for kc in range(DC):
                                nc.tensor.matmul(ps, hb[kc][:, mc * P:(mc + 1) * P],
                                                 wv_t[kc],
                                                 start=(kc == 0), stop=(kc == DC - 1))
                            nc.scalar.activation(
                                out=v_tiles[mc][:, nh * 512:(nh + 1) * 512],
                                in_=ps, func=Act.Copy, scale=invc[mc])
                            yield

                    # ---- attention: software-pipelined head pairs ----
                    att_tiles = [None] * DC

                    def sumav(ti, E, nmp):
                        esum = []
                        for hp in range(2):
                            t1 = ptmp.tile([P, SEQ], BF16, tag="et",
                                           name=f"{nmp}_t1_{ti}{hp}")
                            nc.vector.tensor_add(t1, E[hp][0], E[hp][1])
                            t2 = ptmp.tile([P, SEQ], BF16, tag="et",
                                           name=f"{nmp}_t2_{ti}{hp}")
                            nc.vector.tensor_add(t2, E[hp][2], E[hp][3])
                            t3 = ptmp.tile([P, SEQ], BF16, tag="et",
                                           name=f"{nmp}_t3_{ti}{hp}")
                            nc.vector.tensor_add(t3, t1, t2)
                            esum.append(t3)
                        ps_sb = pp_x.tile([P, SEQ], F32, tag="mmx",
                                          name=f"{nmp}_sb{ti}")
                        for hp in range(2):
                            nc.tensor.matmul(
                                ps_sb[hp * HEAD:(hp + 1) * HEAD, :],
                                ones64_b, esum[hp],
                                start=True, stop=True, skip_group_check=True)
                        rb = prb.tile([P, SEQ], F32, tag="rb", name=f"{nmp}_rb{ti}")
                        nc.vector.reciprocal_approx_fast(out=rb, in_=ps_sb)
                        ps_a = pp_pair.tile([P, SEQ], F32, tag="pair",
                                             name=f"{nmp}_a{ti}")
                        for hp in range(2):
                            hh = 2 * ti + hp
                            for kc in range(TC):
                                nc.tensor.matmul(
                                    ps_a[hp * HEAD:(hp + 1) * HEAD, :],
                                    v_tiles[kc][:, hh * HEAD:(hh + 1) * HEAD],
                                    E[hp][kc],
                                    start=(kc == 0), stop=(kc == TC - 1),
                                    skip_group_check=True)
                        at = patt.tile([P, SEQ], BF16, tag="att",
                                       name=f"{nmp}_at{ti}")
                        nc.vector.tensor_mul(at, ps_a, rb)
                        att_tiles[ti] = at

                    prevE = None
                    for ti in range(DC):
                        # Q/K for this pair from raw hb
                        wtq = pw.tile([P, DC, P], BF16, tag="w",
                                      name=f"{nm}_wq{ti}")
                        nc.sync.dma_start(out=wtq, in_=wqk_d[li, 0, ti])
                        ps_q = pp_x.tile([P, SEQ], F32, tag="mmx",
                                         name=f"{nm}_q{ti}")
                        for kc in range(DC):
                            nc.tensor.matmul(ps_q, wtq[:, kc, :], hb[kc],
                                             start=(kc == 0), stop=(kc == DC - 1))
                        qt = pqk.tile([P, SEQ], BF16, tag="qk", name=f"{nm}_qt{ti}")
                        nc.vector.tensor_mul(qt, ps_q, nb)   # fold inv[q]
                        wtk = pw.tile([P, DC, P], BF16, tag="w",
                                      name=f"{nm}_wk{ti}")
                        nc.sync.dma_start(out=wtk, in_=wqk_d[li, 1, ti])
                        ps_k = pp_s.tile([P, SEQ], F32, tag="mms",
                                         name=f"{nm}_k{ti}")
                        for kc in range(DC):
                            nc.tensor.matmul(ps_k, wtk[:, kc, :], hb[kc],
                                             start=(kc == 0), stop=(kc == DC - 1))
                        kt = pqk.tile([P, SEQ], BF16, tag="qk", name=f"{nm}_kt{ti}")
                        nc.vector.tensor_mul(kt, ps_k, nb)   # fold inv[k]
                        yield

                        # scores + exp (inv[k]*scale applied via ACT scale AP)
                        E = [[], []]
                        for mc in range(TC):
                            for hp in range(2):
                                po = hp * HEAD
                                ps_s = pp_s.tile([P, SEQ], F32, tag="mms",
                                                 name=f"{nm}_s{ti}_{hp}{mc}")
                                nc.tensor.matmul(
                                    ps_s, kt[po:po + HEAD, mc * P:(mc + 1) * P],
                                    qt[po:po + HEAD, :], start=True, stop=True)
                                e = pE.tile([P, SEQ], BF16, tag="E",
                                            name=f"{nm}_e{ti}_{hp}{mc}")
                                nc.scalar.activation(out=e, in_=ps_s, func=Act.Exp,
                                                     scale=scale, bias=zero_col)
                                E[hp].append(e)
                            if mc == 1:
                                yield
                        yield
                        if prevE is not None:
                            sumav(prevE[0], prevE[1], nm)
                            yield
                        prevE = (ti, E)
                    sumav(prevE[0], prevE[1], nm)
                    yield

                    # ---- Wo + residual ----
                    ss_mid = SumSq(nm + "mid")
                    h2 = []
                    for mc in range(DC):
                        wt = pw.tile([P, DC, P], BF16, tag="w", name=f"{nm}_wo{mc}")
                        nc.sync.dma_start(out=wt, in_=wo_d[li, mc])
                        ps = pp_big.tile([P, SEQ], F32, tag="mmb", name=f"{nm}_o{mc}")
                        for kc in range(DC):
                            nc.tensor.matmul(ps, wt[:, kc, :], att_tiles[kc],
                                             start=(kc == 0), stop=(kc == DC - 1))
                        hn = ph.tile([P, SEQ], F32, tag=f"h{s}", name=f"{nm}_h2{mc}")
                        nc.vector.scalar_tensor_tensor(
                            out=hn, in0=ps, scalar=bo_sb[:, mc:mc + 1], in1=h[mc],
                            op0=Alu.add, op1=Alu.add)
                        ss_mid.add(hn, nm + "mid")
                        h2.append(hn)
                        if mc % 2 == 1:
                            yield
                    h = h2

                    # ---------------- FFN ----------------
                    yn = norm_fin(h, ss_mid, nm + "n2")
                    yield
                    g_tiles = []
                    for mc in range(FC):
                        wt = pw.tile([P, DC, P], BF16, tag="w", name=f"{nm}_w1{mc}")
                        nc.sync.dma_start(out=wt, in_=w1_d[li, mc])
                        ps = pp_big.tile([P, SEQ], F32, tag="mmb", name=f"{nm}_f1{mc}")
                        for kc in range(DC):
                            nc.tensor.matmul(ps, wt[:, kc, :], yn[kc],
                                             start=(kc == 0), stop=(kc == DC - 1))
                        g = pg.tile([P, SEQ], BF16, tag="g", name=f"{nm}_g{mc}")
                        nc.scalar.activation(out=g, in_=ps, func=Act.Gelu,
                                             bias=b1_sb[:, mc:mc + 1], scale=1.0)
                        g_tiles.append(g)
                        if mc % 4 == 3:
                            yield

                    last = li == N_LAYER - 1
                    if not last:
                        ss_next = SumSq(nm + "nxt")
                    h3 = []
                    hb = []
                    for mc in range(DC):
                        wt = pw.tile([P, FC, P], BF16, tag="w2", bufs=2,
                                     name=f"{nm}_w2{mc}")
                        nc.sync.dma_start(out=wt, in_=w2_d[li, mc])
                        ps = pp_big.tile([P, SEQ], F32, tag="mmb", name=f"{nm}_f2{mc}")
                        for kc in range(FC):
                            nc.tensor.matmul(ps, wt[:, kc, :], g_tiles[kc],
                                             start=(kc == 0), stop=(kc == FC - 1))
                        hn = ph.tile([P, SEQ], F32, tag=f"h{s}", name=f"{nm}_h3{mc}")
                        nc.vector.scalar_tensor_tensor(
                            out=hn, in0=ps, scalar=b2_sb[:, mc:mc + 1], in1=h[mc],
                            op0=Alu.add, op1=Alu.add)
                        if not last:
                            ss_next.add(hn, nm + "nxt")
                            hbm = pxn.tile([P, SEQ], BF16, tag="xn",
                                           name=f"{nm}_hb{mc}")
                            nc.vector.tensor_copy(out=hbm, in_=hn)
                            hb.append(hbm)
                        h3.append(hn)
                        yield
                    h = h3

                # ---------------- transpose + store ----------------
                for tck in range(TC):
                    ob = posb.tile([P, HIDDEN], F32, tag="osb", name=f"{nm0}_ob{tck}")
                    for dc in range(DC):
                        ps_t = pp_big.tile([P, P], F32, tag="mmb",
                                           name=f"{nm0}_tr{tck}_{dc}")
                        nc.tensor.transpose(ps_t, h[dc][:, tck * P:(tck + 1) * P],
                                            ident)
                        nc.vector.tensor_copy(out=ob[:, dc * P:(dc + 1) * P],
                                              in_=ps_t)
                    nc.sync.dma_start(out=out_d[s, tck * P:(tck + 1) * P, :], in_=ob)
                    yield

            # PE warmup: ~7us of junk matmuls flips the HAM clock gate to
            # 8/8 while the first input DMAs are still in flight
            junk_row = pc.tile([1, SEQ], BF16, name="junk_row")
            nc.vector.memset(junk_row, 1.0)
            for i in range(16):
                ps_w = pp_big.tile([P, SEQ], F32, tag="mmb", name=f"warm{i}")
                nc.tensor.matmul(ps_w, ones_row_b, junk_row,
                                 start=True, stop=True)

            gens = [seq_program(0), seq_program(1)]
            for _ in range(3):           # both embeddings first (PE density)
                next(gens[0])
                next(gens[1])
            for _ in range(OFFSET):
                next(gens[0])
            alive = [True, True]
            while alive[0] or alive[1]:
                for i in range(2):
                    if alive[i]:
                        try:
                            next(gens[i])
                        except StopIteration:
                            alive[i] = False

    from concourse.library_overlay import lower_extended_insts
    lower_extended_insts(nc)   # populate .instr for custom-DVE InstISA ops
    if split_waits:
        _split_multiwait(nc)
    return nc


def _split_multiwait(nc, max_waits=1):
    """This container's walrus accepts at most one sync-wait per instruction;
    hoist excess waits onto standalone EventSemaphore ops on the same engine
    queue (queue order preserves semantics)."""
    import bass_rust
    from bass_rust import SyncInfo

    for fn in nc.m.functions:
        for blk in fn.blocks:
            out = []
            for inst in blk.instructions:
                si = inst.sync_info
                waits = list(si.on_wait) if si is not None and si.on_wait else []
                if len(waits) > max_waits:
                    extra, keep = waits[:-max_waits], waits[-max_waits:]
                    for i, w in enumerate(extra):
                        nop = bass_rust.InstEventSemaphore(
                            name=f"{inst.name}w{i}", engine=inst.engine)
                        nop.sync_info = SyncInfo(on_wait=[w], on_update=[])
                        out.append(nop)
                    inst.sync_info = SyncInfo(
                        on_wait=keep, on_update=list(si.on_update or []))
                out.append(inst)
            blk.instructions = out


def prep_inputs(inputs):
    """Host-side layout prep shared by all cores (weights identical per core)."""
    _ensure_paths()
    import ml_dtypes

    f32 = np.float32
    emb = np.asarray(inputs["emb_table"], f32)       # [32, 1023]
    pos = np.asarray(inputs["pos_table"], f32)       # [512, 1024]
    Wq = np.asarray(inputs["Wq"], f32)               # [6, 16, 1024, 64]
    Wk = np.asarray(inputs["Wk"], f32)
    Wv = np.asarray(inputs["Wv"], f32)
    Wo = np.asarray(inputs["Wo"], f32)               # [6, 1024, 1024]
    W1 = np.asarray(inputs["W1"], f32)               # [6, 1024, 2048]
    W2 = np.asarray(inputs["W2"], f32)               # [6, 2048, 1024]
    g1 = np.asarray(inputs["g1"], f32)               # [6, 1024]
    g2 = np.asarray(inputs["g2"], f32)

    wemb = np.zeros((VOCAB + 1, HIDDEN), ml_dtypes.bfloat16)
    wemb[:VOCAB, :HIDDEN - 1] = emb.astype(ml_dtypes.bfloat16)
    wemb[VOCAB, HIDDEN - 1] = 1.0                    # duration channel
    post = np.ascontiguousarray(pos.T.reshape(DC, P, SEQ))
    iota = np.arange(VOCAB, dtype=f32).reshape(VOCAB, 1)

    def blk_kxm(a, mchunks):
        # [K, M] -> [mc, p, kc, m] blocked for contiguous per-partition DMA
        k, m = a.shape
        return np.ascontiguousarray(
            a.reshape(k // P, P, mchunks, P).transpose(2, 1, 0, 3))

    bf16 = ml_dtypes.bfloat16
    wqk = np.empty((N_LAYER, 2, DC, P, DC, P), bf16)
    wv = np.empty((N_LAYER, DC, P, HIDDEN), bf16)
    wo = np.empty((N_LAYER, DC, P, DC, P), bf16)
    w1 = np.empty((N_LAYER, FC, P, DC, P), bf16)
    w2 = np.empty((N_LAYER, DC, P, FC, P), bf16)
    for i in range(N_LAYER):
        aq = (Wq[i] * g1[i][None, :, None]).transpose(1, 0, 2).reshape(HIDDEN, HIDDEN)
        ak = (Wk[i] * g1[i][None, :, None]).transpose(1, 0, 2).reshape(HIDDEN, HIDDEN)
        av = (Wv[i] * g1[i][None, :, None]).transpose(1, 0, 2).reshape(HIDDEN, HIDDEN)
        wqk[i, 0] = blk_kxm(aq, DC).astype(bf16)
        wqk[i, 1] = blk_kxm(ak, DC).astype(bf16)
        wv[i] = av.reshape(DC, P, HIDDEN).astype(bf16)
        wo[i] = blk_kxm(Wo[i], DC).astype(bf16)
        w1[i] = blk_kxm(g2[i][:, None] * W1[i], FC).astype(bf16)
        w2[i] = blk_kxm(W2[i], DC).astype(bf16)

    base = {
        "wemb": wemb, "post": post, "iota": iota,
        "wqk": wqk, "wv": wv, "wo": wo, "w1": w1, "w2": w2,
        "bo": np.ascontiguousarray(
            np.asarray(inputs["bo"], f32).reshape(N_LAYER, DC, P).transpose(0, 2, 1)),
        "b1": np.ascontiguousarray(
            np.asarray(inputs["b1"], f32).reshape(N_LAYER, FC, P).transpose(0, 2, 1)),
        "b2": np.ascontiguousarray(
            np.asarray(inputs["b2"], f32).reshape(N_LAYER, DC, P).transpose(0, 2, 1)),
    }
    return base


LAST_RESULTS = None


def _ntff_hook():
    """NTFF profiling hook via the axon .so (the concourse<->antenv bridge
    module is absent in this image, so drive the capture directly)."""
    try:
        from trn_agent_boot.trn_boot import _ntff_profile_via_ctypes
        return _ntff_profile_via_ctypes("/opt/axon/libaxon_pjrt.so")
    except Exception as e:
        print("ntff hook unavailable:", e)
        return None


def kernel(**inputs):
    global LAST_RESULTS
    _ensure_paths()
    from concourse.bass_utils import run_bass_kernel_spmd

    x = np.asarray(inputs["x"], np.float32)          # [16, 512, 2]
    base = prep_inputs(inputs)
    in_maps = []
    for c in range(N_CORES):
        m = dict(base)
        m["x"] = np.ascontiguousarray(x[c * SEQ_PER_CORE:(c + 1) * SEQ_PER_CORE])
        in_maps.append(m)

    nc = build_nc()
    trace_dir = os.environ.get("KBENCH_TRACE_DIR")
    if trace_dir:
        hook = _ntff_hook()
        if hook is not None:
            os.makedirs(trace_dir, exist_ok=True)
            with hook(trace_dir, [0]):
                res = run_bass_kernel_spmd(nc, in_maps, list(range(N_CORES)))
        else:
            res = run_bass_kernel_spmd(nc, in_maps, list(range(N_CORES)))
    else:
        res = run_bass_kernel_spmd(nc, in_maps, list(range(N_CORES)))
    LAST_RESULTS = res
    out = np.concatenate(
        [res.results[c]["out"].reshape(SEQ_PER_CORE, SEQ * HIDDEN)
         for c in range(N_CORES)], axis=0)
    return out
